# revision 21
# baseline (speedup 1.0000x reference)
"""Distributed GIN (3-layer) kernel for 8 TRN2 NeuronCores.

Sharding: nodes (and their incident in-edges) are partitioned across the 8
cores; each core keeps its node-feature shard resident in SBUF (transposed,
bf16).  Between layers the per-core shards are AllGathered into a bf16
"gather table" in DRAM; neighbor aggregation is a dma_gather (256B/edge) +
one-hot matmul segment-sum on the tensor engine.  MLP weights are replicated.
Per-graph pooled sums are computed per-core on the PE and reduced on the host.

Perf-critical structure:
- Every (quadrant, dst-block) cell gets exactly CAP=512 gather slots; overflow
  edges go to a per-(quadrant, round) spill stream aggregated with 512-wide
  one-hots.  This keeps dma_gather descriptor count (the Q7/SWDGE serial
  bottleneck) within ~7% of the true edge count.
- Gather calls are per (round, quadrant); the 4 quadrant calls run on the 4
  SWDGE queues, so their Q7 descriptor generation executes on 4 different
  core pairs concurrently (~4x the single-queue gather rate).
- 3-stage software pipeline per round r: gathers(r) / aggregation matmuls
  (r-1) / MLP+emits(r-2), with 4 rounds of gather buffering, so the PE is
  never gated on the ACT z-copy round trip and the gather stream free-runs.
- z = (1+eps)h + agg is accumulated on the PE (eps-scaled identity matmul);
  the whole MLP runs in bf16.
- The gather table is two stacked halves (all cores' rows < HALF_P, then the
  rest) so the per-layer AllGather fires as two contiguous chunks, the first
  mid-layer, overlapping the collective with compute.
"""

import math

import ml_dtypes
import numpy as np

N_NODES = 100000
N_EDGES = 1600000
F = 128              # feature dim (= hidden dim)
L = 3                # GIN layers
G = 64               # graphs
NC = 8               # NeuronCores
PER = N_NODES // NC  # 12500 real nodes per core
BLK = 128
NBLK = (PER + BLK - 1) // BLK          # 98 node blocks per core
PERP = NBLK * BLK                      # 12544 padded rows per core
TROWS = NC * PERP                      # 100352 table rows
NQ = 4
QS = TROWS // NQ                       # 25088 rows per quadrant (< int16 max)
RBLK = 4                               # dst blocks per PSUM round (1 full bank)
NR = (NBLK + RBLK - 1) // RBLK         # 25 rounds
CAP = 512                              # main slots per (q, b) cell (4 tiles)
KC = CAP // BLK                        # 4 tiles per cell
HALF_P = 49 * BLK                      # per-core rows in each AllGather chunk

_F32 = np.float32
_BF16 = ml_dtypes.bfloat16


def _table_row(node):
    """Global node id -> row in the (padded) all-gathered table.

    Table layout is two stacked halves so chunked AllGathers write
    contiguous slices: [all cores' rows 0:HALF_P][all cores' rows HALF_P:].
    """
    c = node // PER
    p = node % PER
    lo = p < HALF_P
    return np.where(lo, c * HALF_P + p,
                    NC * HALF_P + c * HALF_P + (p - HALF_P))


def _round_blocks(r):
    b0 = r * RBLK
    return range(b0, min(b0 + RBLK, NBLK))


def _build_structures(edge_index, batch):
    """Host-side preprocessing: per-core padded edge-slot streams.

    Layout per (r, q) gather call:
      [cell (q,b) : 512 slots for b in blocks(r)] ++
      [spill (q,r) : Ksp[q][r]*128 slots]
    """
    src = np.asarray(edge_index[0], dtype=np.int64)
    dst = np.asarray(edge_index[1], dtype=np.int64)
    srow = _table_row(src)
    q_of_edge = srow // QS
    qloc = (srow % QS).astype(np.int16)

    core_of_edge = dst // PER
    dst_local = dst % PER
    b_of_edge = dst_local // BLK

    # ---- per-core main/spill split (cap CAP per cell) ----
    per_core = []
    spill_counts = np.zeros((NC, NQ, NR), dtype=np.int64)
    for c in range(NC):
        sel = np.nonzero(core_of_edge == c)[0]
        eq = q_of_edge[sel]
        eb = b_of_edge[sel]
        order = np.lexsort((qloc[sel], eb, eq))
        sel = sel[order]
        eq, eb = eq[order], eb[order]
        gid = eq * NBLK + eb
        run_starts = np.zeros(NQ * NBLK + 1, dtype=np.int64)
        np.add.at(run_starts, gid + 1, 1)
        run_starts = np.cumsum(run_starts)
        rank = np.arange(gid.size) - run_starts[gid]
        is_spill = rank >= CAP
        er = eb // RBLK
        np.add.at(spill_counts, (np.full(sel.size, c)[is_spill],
                                 eq[is_spill], er[is_spill]), 1)
        per_core.append((sel, eq, eb, rank, is_spill))

    Ksp = np.ceil(spill_counts.max(axis=0) / BLK).astype(np.int64)  # [NQ, NR]

    # ---- static slot-stream layout: r-major, then q ----
    cell_tile = {}    # (q, b) -> global tile index of cell's first tile
    spill_tile = {}   # (q, r) -> global tile index of spill's first tile
    sp_packed = {}    # (q, r) -> packed spill-tile index (for the drsp plane)
    call_meta = []    # per (r, q): (slot_offset, n_slots)
    off = 0
    spk = 0
    for r in range(NR):
        for q in range(NQ):
            call_off = off
            for b in _round_blocks(r):
                cell_tile[(q, b)] = off // BLK
                off += CAP
            spill_tile[(q, r)] = off // BLK
            sp_packed[(q, r)] = spk
            off += int(Ksp[q, r]) * BLK
            spk += int(Ksp[q, r])
            call_meta.append((r, q, call_off, off - call_off))
    tot_slots = off
    tot_tiles = tot_slots // BLK
    n_sp_tiles = max(int(Ksp.sum()), 1)

    idx_planes = []
    drel_planes = []
    drsp_planes = []
    bg_planes = []
    flat_cell = np.zeros(NQ * NBLK, dtype=np.int64)
    for (q, b), t in cell_tile.items():
        flat_cell[q * NBLK + b] = t * BLK
    flat_sp = np.zeros(NQ * NR, dtype=np.int64)
    flat_sp_packed = np.zeros(NQ * NR, dtype=np.int64)
    for (q, r), t in spill_tile.items():
        flat_sp[q * NR + r] = t * BLK
        flat_sp_packed[q * NR + r] = sp_packed[(q, r)] * BLK
    for c in range(NC):
        sel, eq, eb, rank, is_spill = per_core[c]
        idx = np.zeros(tot_slots, dtype=np.int16)
        drel = np.full(tot_slots, -1.0, dtype=np.float32)
        drsp = np.full(n_sp_tiles * BLK, -1.0, dtype=np.float32)

        # main slots
        m = ~is_spill
        gid_m = eq[m] * NBLK + eb[m]
        pos_m = flat_cell[gid_m] + rank[m]
        idx[pos_m] = qloc[sel[m]]
        drel[pos_m] = (dst_local[sel[m]] - eb[m] * BLK).astype(np.float32)

        # spill slots: per (q, r) ordered by srow
        s = is_spill
        if s.any():
            eqs = eq[s]
            ers = eb[s] // RBLK
            srt = np.lexsort((qloc[sel[s]], ers, eqs))
            eqs, ers = eqs[srt], ers[srt]
            sel_s = sel[s][srt]
            gid_s = eqs * NR + ers
            rs = np.zeros(NQ * NR + 1, dtype=np.int64)
            np.add.at(rs, gid_s + 1, 1)
            rs = np.cumsum(rs)
            rank_s = np.arange(gid_s.size) - rs[gid_s]
            idx[flat_sp[gid_s] + rank_s] = qloc[sel_s]
            drsp[flat_sp_packed[gid_s] + rank_s] = (
                dst_local[sel_s] - ers * RBLK * BLK).astype(np.float32)

        idx_plane = np.tile(idx.reshape(-1, 16).T, (8, 1))          # [128, tot/16]
        drel_plane = drel.reshape(-1, BLK).T.astype(_BF16)          # [128, tot_tiles]
        drsp_plane = drsp.reshape(-1, BLK).T.astype(_F32)           # [128, n_sp_tiles]
        idx_planes.append(np.ascontiguousarray(idx_plane))
        drel_planes.append(np.ascontiguousarray(drel_plane))
        drsp_planes.append(np.ascontiguousarray(drsp_plane))

        # per-node graph ids (bf16 values; -1 pad), for on-the-fly pool one-hots
        bgfull = np.full(NBLK * BLK, -1.0, dtype=np.float32)
        bgfull[:PER] = np.asarray(batch[c * PER:(c + 1) * PER], dtype=np.float32)
        bg_planes.append(np.ascontiguousarray(
            bgfull.reshape(NBLK, BLK).T.astype(_BF16)))

    return {
        "Ksp": Ksp,
        "cell_tile": cell_tile,
        "spill_tile": spill_tile,
        "sp_packed": sp_packed,
        "call_meta": call_meta,
        "tot_slots": tot_slots,
        "tot_tiles": tot_tiles,
        "n_sp_tiles": n_sp_tiles,
        "idx_planes": idx_planes,
        "drel_planes": drel_planes,
        "drsp_planes": drsp_planes,
        "bg_planes": bg_planes,
    }


def _build_program(struct, eps_vals):
    import concourse.bacc as bacc
    import concourse.mybir as mybir
    from concourse import tile

    dt = mybir.dt
    AF = mybir.ActivationFunctionType
    OP = mybir.AluOpType

    tot_slots = struct["tot_slots"]
    tot_tiles = struct["tot_tiles"]
    n_sp_tiles = struct["n_sp_tiles"]
    call_meta = struct["call_meta"]
    cell_tile = struct["cell_tile"]
    spill_tile = struct["spill_tile"]
    sp_packed = struct["sp_packed"]
    Ksp = struct["Ksp"]
    OHT = KC * RBLK  # 16 main one-hot tiles per (q, round)

    nc = bacc.Bacc("TRN2", target_bir_lowering=False, num_swdge_queues=4)

    # ---- kernel parameters (per-core values via in_maps) ----
    xT_p = nc.declare_dram_parameter("xTb", [F, PERP], dt.bfloat16, isOutput=False)
    idx_p = nc.declare_dram_parameter("idx", [128, tot_slots // 16], dt.int16, isOutput=False)
    drel_p = nc.declare_dram_parameter("dstrel", [128, tot_tiles], dt.bfloat16, isOutput=False)
    drsp_p = nc.declare_dram_parameter("dstrelsp", [128, n_sp_tiles], dt.float32, isOutput=False)
    bg_p = nc.declare_dram_parameter("bg", [128, NBLK], dt.bfloat16, isOutput=False)
    iota2_p = nc.declare_dram_parameter("iota2", [128, 128 * OHT], dt.bfloat16, isOutput=False)
    iota512_p = nc.declare_dram_parameter("iota512", [128, 512], dt.float32, isOutput=False)
    ident_p = nc.declare_dram_parameter("ident", [128, 128], dt.bfloat16, isOutput=False)
    Wp1_p = nc.declare_dram_parameter("Wp1b", [F, F], dt.bfloat16, isOutput=False)
    bp1_p = nc.declare_dram_parameter("bp1", [F, 1], dt.float32, isOutput=False)
    Wp2_p = nc.declare_dram_parameter("Wp2b", [F, F], dt.bfloat16, isOutput=False)
    bp2_p = nc.declare_dram_parameter("bp2", [F, 1], dt.float32, isOutput=False)
    W1_p = nc.declare_dram_parameter("W1sb", [L, F, F], dt.bfloat16, isOutput=False)
    b1_p = nc.declare_dram_parameter("b1s", [L, F, 1], dt.float32, isOutput=False)
    W2_p = nc.declare_dram_parameter("W2sb", [L, F, F], dt.bfloat16, isOutput=False)
    b2_p = nc.declare_dram_parameter("b2s", [L, F, 1], dt.float32, isOutput=False)
    out_p = nc.declare_dram_parameter("out", [G, L * F], dt.float32, isOutput=True)

    # ---- internal DRAM ----
    ag_in = nc.dram_tensor("ag_in", [PERP, F], dt.bfloat16)
    tables = [
        nc.dram_tensor(f"table{l}", [TROWS, F], dt.bfloat16, addr_space="Shared")
        for l in range(L)
    ]

    with tile.TileContext(nc) as tc:
        with (
            tc.tile_pool(name="const", bufs=1) as cpool,
            tc.tile_pool(name="ht", bufs=1) as hpool,
            tc.tile_pool(name="gath", bufs=20) as gpool,
            tc.tile_pool(name="idxp", bufs=20) as ipool,
            tc.tile_pool(name="oh", bufs=12) as ohpool,
            tc.tile_pool(name="ohsp", bufs=4) as ohsp_pool,
            tc.tile_pool(name="zz", bufs=2) as zpool,
            tc.tile_pool(name="emit", bufs=4) as epool,
            tc.tile_pool(name="psag", bufs=4, space="PSUM") as psag,
            tc.tile_pool(name="psmlp", bufs=2, space="PSUM") as psmlp,
            tc.tile_pool(name="pstr", bufs=1, space="PSUM") as pstr,
            tc.tile_pool(name="pspool", bufs=1, space="PSUM") as pspool,
        ):
            # ---- load constants / weights ----
            iota2_sb = cpool.tile([128, 128, OHT], dt.bfloat16, tag="iota2")
            nc.sync.dma_start(iota2_sb[:].rearrange("p a b -> p (a b)"), iota2_p[:])
            iota512_sb = cpool.tile([128, 512], dt.float32, tag="iota512")
            nc.sync.dma_start(iota512_sb[:], iota512_p[:])
            ident_sb = cpool.tile([128, 128], dt.bfloat16, tag="ident")
            nc.sync.dma_start(ident_sb[:], ident_p[:])
            bg_sb = cpool.tile([128, NBLK], dt.bfloat16, tag="bg")
            nc.sync.dma_start(bg_sb[:], bg_p[:])
            drel_sb = cpool.tile([128, tot_tiles], dt.bfloat16, tag="drel")
            nc.sync.dma_start(drel_sb[:], drel_p[:])
            drsp_sb = cpool.tile([128, n_sp_tiles], dt.float32, tag="drsp")
            nc.sync.dma_start(drsp_sb[:], drsp_p[:])

            def _load_w(tag, pslice):
                t = cpool.tile([F, F], dt.bfloat16, tag=tag)
                nc.sync.dma_start(t[:], pslice)
                return t

            def _load_b(tag, pslice):
                t = cpool.tile([F, 1], dt.float32, tag=tag)
                nc.sync.dma_start(t[:], pslice)
                return t

            Wp1 = _load_w("Wp1", Wp1_p[:])
            Wp2 = _load_w("Wp2", Wp2_p[:])
            bp1 = _load_b("bp1", bp1_p[:])
            bp2 = _load_b("bp2", bp2_p[:])
            W1 = [_load_w(f"W1_{l}", W1_p[l][:]) for l in range(L)]
            W2 = [_load_w(f"W2_{l}", W2_p[l][:]) for l in range(L)]
            b1 = [_load_b(f"b1_{l}", b1_p[l][:]) for l in range(L)]
            b2 = [_load_b(f"b2_{l}", b2_p[l][:]) for l in range(L)]
            # eps-scaled identities: z-matmul lhsT, one per layer
            epsI = []
            for l in range(L):
                t = cpool.tile([128, 128], dt.bfloat16, tag=f"epsI{l}")
                nc.vector.tensor_scalar(t[:], ident_sb[:],
                                        float(1.0 + eps_vals[l]), None,
                                        op0=OP.mult)
                epsI.append(t)

            hT = hpool.tile([F, PERP], dt.bfloat16, tag="hT")

            # ---- pre-MLP: hT = relu(relu(x Wp1 + bp1) Wp2 + bp2), transposed ----
            MLP_CHUNK = 512
            o = 0
            while o < PERP:
                cw = min(MLP_CHUNK, PERP - o)
                xc = zpool.tile([F, MLP_CHUNK], dt.bfloat16, tag="xc")
                nc.sync.dma_start(xc[:, :cw], xT_p[:, o:o + cw])
                p1 = psmlp.tile([F, MLP_CHUNK], dt.float32, tag="mlp")
                nc.tensor.matmul(p1[:, :cw], Wp1[:], xc[:, :cw])
                t1 = zpool.tile([F, MLP_CHUNK], dt.bfloat16, tag="t1")
                nc.scalar.activation(t1[:, :cw], p1[:, :cw], AF.Relu, bias=bp1[:])
                p2 = psmlp.tile([F, MLP_CHUNK], dt.float32, tag="mlp")
                nc.tensor.matmul(p2[:, :cw], Wp2[:], t1[:, :cw])
                nc.scalar.activation(hT[:, o:o + cw], p2[:, :cw], AF.Relu, bias=bp2[:])
                o += cw

            pool_psums = []

            def _emit_block(b, layer_out):
                """Transpose block b of hT; DMA to ag_in (if a table is still
                needed) and accumulate pooling (if layer_out >= 1)."""
                ptr = pstr.tile([128, 128], dt.bfloat16, tag="tr")
                nc.tensor.transpose(ptr[:], hT[:, b * BLK:(b + 1) * BLK],
                                    ident_sb[:])
                hrow = epool.tile([128, 128], dt.bfloat16, tag="hrow")
                nc.scalar.activation(hrow[:], ptr[:], AF.Copy)
                if layer_out < L:
                    nc.sync.dma_start(ag_in[b * BLK:(b + 1) * BLK, :], hrow[:])
                if layer_out >= 1:
                    ohg = epool.tile([128, G], dt.bfloat16, tag="ohg")
                    nc.vector.tensor_tensor(
                        ohg[:],
                        bg_sb[:, b:b + 1].broadcast_to([128, G]),
                        iota2_sb[:, :G, 0],
                        OP.is_equal,
                    )
                    nc.tensor.matmul(
                        pool_psums[layer_out - 1][:],
                        ohg[:],
                        hrow[:],
                        start=(b == 0),
                        stop=(b == NBLK - 1),
                        skip_group_check=True,
                    )

            def _ag_chunk(l_out, lo, hi):
                """AllGather ag_in[lo:hi] into the matching contiguous table
                half (table layout is [cores x rows<HALF_P][cores x rest])."""
                base = 0 if lo == 0 else NC * HALF_P
                nc.gpsimd.collective_compute(
                    "AllGather", OP.bypass,
                    replica_groups=[list(range(NC))],
                    ins=[ag_in[lo:hi, :]],
                    outs=[tables[l_out][base:base + NC * (hi - lo), :]],
                )

            # emit h0 -> table0 (AllGather fired in 2 chunks to overlap)
            for b in range(NBLK):
                _emit_block(b, 0)
                if b == HALF_P // BLK - 1:
                    _ag_chunk(0, 0, HALF_P)
            _ag_chunk(0, HALF_P, PERP)

            # ---- GIN layers: 3-stage pipeline over rounds ----
            for l in range(L):
                pool_psums.append(pspool.tile([G, F], dt.float32, tag="pool",
                                              name=f"poolp{l}"))

                def _issue_gathers(r):
                    gts = {}
                    for (rr, q, call_off, n_slots) in call_meta:
                        if rr != r or n_slots == 0:
                            continue
                        idxs = ipool.tile([128, n_slots // 16], dt.int16,
                                          tag="idxs", name=f"idxs_l{l}_r{r}_q{q}")
                        nc.sync.dma_start(
                            idxs[:], idx_p[:, call_off // 16:(call_off + n_slots) // 16]
                        )
                        gt = gpool.tile([128, n_slots // BLK, 128], dt.bfloat16,
                                        tag="gt", name=f"gt_l{l}_r{r}_q{q}")
                        nc.gpsimd.dma_gather(
                            gt[:],
                            tables[l][q * QS:(q + 1) * QS, :],
                            idxs[:],
                            n_slots,
                            n_slots,
                            F,
                            single_packet=False,
                            queue_num=q,
                        )
                        gts[q] = (gt, call_off // BLK)
                    return gts

                def _aggregate(r, gts):
                    rblocks = [b for b in _round_blocks(r)]
                    cw = len(rblocks) * BLK
                    o = rblocks[0] * BLK
                    agg_r = psag.tile([F, RBLK * BLK], dt.float32, tag="agg",
                                      name=f"agg_l{l}_r{r}")
                    # start=True resets the whole PSUM bank, so windows can't
                    # each open their own group: zero once, all-accumulate.
                    nc.vector.memset(agg_r[:], 0.0)
                    for q in range(NQ):
                        t0 = cell_tile[(q, rblocks[0])]
                        Tr = KC * len(rblocks)
                        oh = ohpool.tile([128, 128, OHT], dt.bfloat16, tag="oh",
                                         name=f"oh_l{l}_r{r}_q{q}")
                        nc.vector.tensor_tensor(
                            oh[:, :, :Tr],
                            drel_sb[:, t0:t0 + Tr].unsqueeze(1)
                                .broadcast_to([128, 128, Tr]),
                            iota2_sb[:, :, :Tr],
                            OP.is_equal,
                        )
                        gt, c0 = gts[q]
                        for w, b in enumerate(rblocks):
                            bt = cell_tile[(q, b)]
                            for t in range(KC):
                                nc.tensor.matmul(
                                    agg_r[:, w * BLK:(w + 1) * BLK],
                                    gt[:, bt - c0 + t, :],
                                    oh[:, :, bt - t0 + t],
                                    start=False,
                                    stop=False,
                                    skip_group_check=True,
                                )
                        # spill matmuls: 512-wide one-hots over the full bank
                        ksp = int(Ksp[q, r])
                        st0 = spill_tile[(q, r)]
                        sp0 = sp_packed[(q, r)]
                        for t in range(ksp):
                            ohsp = ohsp_pool.tile([128, 512], dt.bfloat16,
                                                  tag="ohsp",
                                                  name=f"ohsp_l{l}_r{r}_q{q}_{t}")
                            nc.vector.tensor_tensor(
                                ohsp[:],
                                drsp_sb[:, sp0 + t:sp0 + t + 1]
                                    .broadcast_to([128, 512]),
                                iota512_sb[:],
                                OP.is_equal,
                            )
                            nc.tensor.matmul(
                                agg_r[:],
                                gt[:, st0 - c0 + t, :],
                                ohsp[:],
                                start=False,
                                stop=False,
                                skip_group_check=True,
                            )
                    # z = agg + (1+eps) h via eps-scaled identity matmul
                    nc.tensor.matmul(
                        agg_r[:, :cw],
                        epsI[l][:],
                        hT[:, o:o + cw],
                        start=False,
                        stop=True,
                        skip_group_check=True,
                    )
                    return (o, cw, agg_r)

                def _mlp_chunk(o, cw, agg_r):
                    # z was accumulated in PSUM: (1+eps)h + agg; copy to SBUF bf16
                    z = zpool.tile([F, MLP_CHUNK], dt.bfloat16, tag="z",
                                   name=f"z_l{l}_o{o}")
                    nc.scalar.activation(z[:, :cw], agg_r[:, :cw], AF.Copy)
                    p1 = psmlp.tile([F, MLP_CHUNK], dt.float32, tag="mlp",
                                    name=f"p1_l{l}_o{o}")
                    nc.tensor.matmul(p1[:, :cw], W1[l][:], z[:, :cw])
                    t1 = zpool.tile([F, MLP_CHUNK], dt.bfloat16, tag="t1",
                                    name=f"t1_l{l}_o{o}")
                    nc.scalar.activation(t1[:, :cw], p1[:, :cw], AF.Relu, bias=b1[l][:])
                    p2 = psmlp.tile([F, MLP_CHUNK], dt.float32, tag="mlp",
                                    name=f"p2_l{l}_o{o}")
                    nc.tensor.matmul(p2[:, :cw], W2[l][:], t1[:, :cw])
                    nc.scalar.activation(hT[:, o:o + cw], p2[:, :cw], AF.Identity,
                                         bias=b2[l][:])
                    for k in range(cw // BLK):
                        _emit_block(o // BLK + k, l + 1)

                # pipeline: gathers(r) | aggregate(r-1) | mlp+emit(r-2)
                gts_q = {}
                agg_q = {}
                ag1_round = (HALF_P // BLK) // RBLK  # mlp of this round
                for r in range(NR + 2):
                    if r < NR:
                        gts_q[r] = _issue_gathers(r)
                    if 0 <= r - 1 < NR:
                        agg_q[r - 1] = _aggregate(r - 1, gts_q.pop(r - 1))
                    if 0 <= r - 2:
                        _mlp_chunk(*agg_q.pop(r - 2))
                        if l + 1 < L and r - 2 == ag1_round:
                            _ag_chunk(l + 1, 0, HALF_P)

                if l + 1 < L:
                    _ag_chunk(l + 1, HALF_P, PERP)
                # extract pooled sums for this layer
                pooled_sb = epool.tile([G, F], dt.float32, tag="pooled")
                nc.scalar.activation(pooled_sb[:], pool_psums[l][:], AF.Copy)
                nc.sync.dma_start(out_p[:, l * F:(l + 1) * F], pooled_sb[:])

    nc.compile()
    return nc


def _make_in_maps(struct, inputs):
    x = np.asarray(inputs["x"], dtype=_F32)
    OHT = KC * RBLK
    iota2 = np.broadcast_to(
        np.arange(128, dtype=_F32)[:, None], (128, 128, OHT)
    ).reshape(128, 128 * OHT).astype(_BF16)  # iota2[p, j*OHT + t] = j
    iota512 = np.ascontiguousarray(np.broadcast_to(
        np.arange(512, dtype=_F32), (128, 512)))
    ident = np.eye(128, dtype=_F32).astype(_BF16)

    shared = {
        "iota2": np.ascontiguousarray(iota2),
        "iota512": iota512,
        "ident": np.ascontiguousarray(ident),
        "Wp1b": np.asarray(inputs["W_pre1"]).astype(_BF16),
        "bp1": np.asarray(inputs["b_pre1"], dtype=_F32).reshape(F, 1),
        "Wp2b": np.asarray(inputs["W_pre2"]).astype(_BF16),
        "bp2": np.asarray(inputs["b_pre2"], dtype=_F32).reshape(F, 1),
        "W1sb": np.asarray(inputs["W1s"]).astype(_BF16),
        "b1s": np.asarray(inputs["b1s"], dtype=_F32).reshape(L, F, 1),
        "W2sb": np.asarray(inputs["W2s"]).astype(_BF16),
        "b2s": np.asarray(inputs["b2s"], dtype=_F32).reshape(L, F, 1),
    }

    in_maps = []
    for c in range(NC):
        xs = np.zeros((F, PERP), dtype=_F32)
        xs[:, :PER] = x[c * PER:(c + 1) * PER].T
        m = dict(shared)
        m["xTb"] = xs.astype(_BF16)
        m["idx"] = struct["idx_planes"][c]
        m["dstrel"] = struct["drel_planes"][c]
        m["dstrelsp"] = struct["drsp_planes"][c]
        m["bg"] = struct["bg_planes"][c]
        in_maps.append(m)
    return in_maps


def kernel(**inputs):
    from concourse.bass_utils import run_bass_kernel_spmd

    edge_index = np.asarray(inputs["edge_index"])
    batch = np.asarray(inputs["batch"])
    eps = np.asarray(inputs["eps"], dtype=_F32)

    struct = _build_structures(edge_index, batch)
    nc = _build_program(struct, [float(e) for e in eps])
    in_maps = _make_in_maps(struct, inputs)

    res = run_bass_kernel_spmd(nc, in_maps, core_ids=list(range(NC)))
    out = np.zeros((G, L * F), dtype=_F32)
    for c in range(NC):
        out += res.results[c]["out"]
    return out


# revision 25
# speedup vs baseline: 1.0028x; 1.0028x over previous
"""Distributed GIN (3-layer) kernel for 8 TRN2 NeuronCores.

Sharding: nodes (and their incident in-edges) are partitioned across the 8
cores; each core keeps its node-feature shard resident in SBUF (transposed,
bf16).  Between layers the per-core shards are AllGathered into a bf16
"gather table" in DRAM; neighbor aggregation is a dma_gather (256B/edge) +
one-hot matmul segment-sum on the tensor engine.  MLP weights are replicated.
Per-graph pooled sums are computed per-core on the PE and reduced on the host.

Perf-critical structure:
- Every (quadrant, dst-block) cell gets exactly CAP=512 gather slots; overflow
  edges go to a per-(quadrant, round) spill stream aggregated with 512-wide
  one-hots.  This keeps dma_gather descriptor count (the Q7/SWDGE serial
  bottleneck) within ~7% of the true edge count.
- Gather calls are per (round, quadrant); the 4 quadrant calls run on the 4
  SWDGE queues, so their Q7 descriptor generation executes on 4 different
  core pairs concurrently (~4x the single-queue gather rate).
- 3-stage software pipeline per round r: gathers(r) / aggregation matmuls
  (r-1) / MLP+emits(r-2), with 4 rounds of gather buffering, so the PE is
  never gated on the ACT z-copy round trip and the gather stream free-runs.
- z = (1+eps)h + agg is accumulated on the PE (eps-scaled identity matmul);
  the whole MLP runs in bf16.
- The gather table is two stacked halves (all cores' rows < HALF_P, then the
  rest) so the per-layer AllGather fires as two contiguous chunks, the first
  mid-layer, overlapping the collective with compute.
"""

import math

import ml_dtypes
import numpy as np

N_NODES = 100000
N_EDGES = 1600000
F = 128              # feature dim (= hidden dim)
L = 3                # GIN layers
G = 64               # graphs
NC = 8               # NeuronCores
PER = N_NODES // NC  # 12500 real nodes per core
BLK = 128
NBLK = (PER + BLK - 1) // BLK          # 98 node blocks per core
PERP = NBLK * BLK                      # 12544 padded rows per core
TROWS = NC * PERP                      # 100352 table rows
NQ = 4
QS = TROWS // NQ                       # 25088 rows per quadrant (< int16 max)
RBLK = 4                               # dst blocks per PSUM round (1 full bank)
NR = (NBLK + RBLK - 1) // RBLK         # 25 rounds
CAP = 512                              # main slots per (q, b) cell (4 tiles)
KC = CAP // BLK                        # 4 tiles per cell
HALF_P = 49 * BLK                      # per-core rows in each AllGather chunk

_F32 = np.float32
_BF16 = ml_dtypes.bfloat16


def _table_row(node):
    """Global node id -> row in the (padded) all-gathered table.

    Table layout is two stacked halves so chunked AllGathers write
    contiguous slices: [all cores' rows 0:HALF_P][all cores' rows HALF_P:].
    """
    c = node // PER
    p = node % PER
    lo = p < HALF_P
    return np.where(lo, c * HALF_P + p,
                    NC * HALF_P + c * HALF_P + (p - HALF_P))


def _round_blocks(r):
    b0 = r * RBLK
    return range(b0, min(b0 + RBLK, NBLK))


def _build_structures(edge_index, batch):
    """Host-side preprocessing: per-core padded edge-slot streams.

    Layout per (r, q) gather call:
      [cell (q,b) : 512 slots for b in blocks(r)] ++
      [spill (q,r) : Ksp[q][r]*128 slots]
    """
    src = np.asarray(edge_index[0], dtype=np.int64)
    dst = np.asarray(edge_index[1], dtype=np.int64)
    srow = _table_row(src)
    q_of_edge = srow // QS
    qloc = (srow % QS).astype(np.int16)

    core_of_edge = dst // PER
    dst_local = dst % PER
    b_of_edge = dst_local // BLK

    # ---- per-core main/spill split (cap CAP per cell) ----
    per_core = []
    spill_counts = np.zeros((NC, NQ, NR), dtype=np.int64)
    for c in range(NC):
        sel = np.nonzero(core_of_edge == c)[0]
        eq = q_of_edge[sel]
        eb = b_of_edge[sel]
        order = np.lexsort((qloc[sel], eb, eq))
        sel = sel[order]
        eq, eb = eq[order], eb[order]
        gid = eq * NBLK + eb
        run_starts = np.zeros(NQ * NBLK + 1, dtype=np.int64)
        np.add.at(run_starts, gid + 1, 1)
        run_starts = np.cumsum(run_starts)
        rank = np.arange(gid.size) - run_starts[gid]
        is_spill = rank >= CAP
        er = eb // RBLK
        np.add.at(spill_counts, (np.full(sel.size, c)[is_spill],
                                 eq[is_spill], er[is_spill]), 1)
        per_core.append((sel, eq, eb, rank, is_spill))

    Ksp = np.ceil(spill_counts.max(axis=0) / BLK).astype(np.int64)  # [NQ, NR]

    # ---- static slot-stream layout: r-major, then q ----
    cell_tile = {}    # (q, b) -> global tile index of cell's first tile
    spill_tile = {}   # (q, r) -> global tile index of spill's first tile
    sp_packed = {}    # (q, r) -> packed spill-tile index (for the drsp plane)
    call_meta = []    # per (r, q): (slot_offset, n_slots)
    off = 0
    spk = 0
    for r in range(NR):
        for q in range(NQ):
            call_off = off
            for b in _round_blocks(r):
                cell_tile[(q, b)] = off // BLK
                off += CAP
            spill_tile[(q, r)] = off // BLK
            sp_packed[(q, r)] = spk
            off += int(Ksp[q, r]) * BLK
            spk += int(Ksp[q, r])
            call_meta.append((r, q, call_off, off - call_off))
    tot_slots = off
    tot_tiles = tot_slots // BLK
    n_sp_tiles = max(int(Ksp.sum()), 1)

    idx_planes = []
    drel_planes = []
    drsp_planes = []
    bg_planes = []
    flat_cell = np.zeros(NQ * NBLK, dtype=np.int64)
    for (q, b), t in cell_tile.items():
        flat_cell[q * NBLK + b] = t * BLK
    flat_sp = np.zeros(NQ * NR, dtype=np.int64)
    flat_sp_packed = np.zeros(NQ * NR, dtype=np.int64)
    for (q, r), t in spill_tile.items():
        flat_sp[q * NR + r] = t * BLK
        flat_sp_packed[q * NR + r] = sp_packed[(q, r)] * BLK
    for c in range(NC):
        sel, eq, eb, rank, is_spill = per_core[c]
        idx = np.zeros(tot_slots, dtype=np.int16)
        drel = np.full(tot_slots, -1.0, dtype=np.float32)
        drsp = np.full(n_sp_tiles * BLK, -1.0, dtype=np.float32)

        # main slots
        m = ~is_spill
        gid_m = eq[m] * NBLK + eb[m]
        pos_m = flat_cell[gid_m] + rank[m]
        idx[pos_m] = qloc[sel[m]]
        drel[pos_m] = (dst_local[sel[m]] - eb[m] * BLK).astype(np.float32)

        # spill slots: per (q, r) ordered by srow
        s = is_spill
        if s.any():
            eqs = eq[s]
            ers = eb[s] // RBLK
            srt = np.lexsort((qloc[sel[s]], ers, eqs))
            eqs, ers = eqs[srt], ers[srt]
            sel_s = sel[s][srt]
            gid_s = eqs * NR + ers
            rs = np.zeros(NQ * NR + 1, dtype=np.int64)
            np.add.at(rs, gid_s + 1, 1)
            rs = np.cumsum(rs)
            rank_s = np.arange(gid_s.size) - rs[gid_s]
            idx[flat_sp[gid_s] + rank_s] = qloc[sel_s]
            drsp[flat_sp_packed[gid_s] + rank_s] = (
                dst_local[sel_s] - ers * RBLK * BLK).astype(np.float32)

        idx_plane = np.tile(idx.reshape(-1, 16).T, (8, 1))          # [128, tot/16]
        drel_plane = drel.reshape(-1, BLK).T.astype(_BF16)          # [128, tot_tiles]
        drsp_plane = drsp.reshape(-1, BLK).T.astype(_F32)           # [128, n_sp_tiles]
        idx_planes.append(np.ascontiguousarray(idx_plane))
        drel_planes.append(np.ascontiguousarray(drel_plane))
        drsp_planes.append(np.ascontiguousarray(drsp_plane))

        # per-node graph ids (bf16 values; -1 pad), for on-the-fly pool one-hots
        bgfull = np.full(NBLK * BLK, -1.0, dtype=np.float32)
        bgfull[:PER] = np.asarray(batch[c * PER:(c + 1) * PER], dtype=np.float32)
        bg_planes.append(np.ascontiguousarray(
            bgfull.reshape(NBLK, BLK).T.astype(_BF16)))

    return {
        "Ksp": Ksp,
        "cell_tile": cell_tile,
        "spill_tile": spill_tile,
        "sp_packed": sp_packed,
        "call_meta": call_meta,
        "tot_slots": tot_slots,
        "tot_tiles": tot_tiles,
        "n_sp_tiles": n_sp_tiles,
        "idx_planes": idx_planes,
        "drel_planes": drel_planes,
        "drsp_planes": drsp_planes,
        "bg_planes": bg_planes,
    }


def _build_program(struct, eps_vals):
    import concourse.bacc as bacc
    import concourse.mybir as mybir
    from concourse import tile

    dt = mybir.dt
    AF = mybir.ActivationFunctionType
    OP = mybir.AluOpType

    tot_slots = struct["tot_slots"]
    tot_tiles = struct["tot_tiles"]
    n_sp_tiles = struct["n_sp_tiles"]
    call_meta = struct["call_meta"]
    cell_tile = struct["cell_tile"]
    spill_tile = struct["spill_tile"]
    sp_packed = struct["sp_packed"]
    Ksp = struct["Ksp"]
    OHT = KC * RBLK  # 16 main one-hot tiles per (q, round)

    nc = bacc.Bacc("TRN2", target_bir_lowering=False, num_swdge_queues=4)

    # ---- kernel parameters (per-core values via in_maps) ----
    xT_p = nc.declare_dram_parameter("xTb", [F, PERP], dt.bfloat16, isOutput=False)
    idx_p = nc.declare_dram_parameter("idx", [128, tot_slots // 16], dt.int16, isOutput=False)
    drel_p = nc.declare_dram_parameter("dstrel", [128, tot_tiles], dt.bfloat16, isOutput=False)
    drsp_p = nc.declare_dram_parameter("dstrelsp", [128, n_sp_tiles], dt.float32, isOutput=False)
    bg_p = nc.declare_dram_parameter("bg", [128, NBLK], dt.bfloat16, isOutput=False)
    iota2_p = nc.declare_dram_parameter("iota2", [128, 128 * OHT], dt.bfloat16, isOutput=False)
    iota512_p = nc.declare_dram_parameter("iota512", [128, 512], dt.float32, isOutput=False)
    ident_p = nc.declare_dram_parameter("ident", [128, 128], dt.bfloat16, isOutput=False)
    Wp1_p = nc.declare_dram_parameter("Wp1b", [F, F], dt.bfloat16, isOutput=False)
    bp1_p = nc.declare_dram_parameter("bp1", [F, 1], dt.float32, isOutput=False)
    Wp2_p = nc.declare_dram_parameter("Wp2b", [F, F], dt.bfloat16, isOutput=False)
    bp2_p = nc.declare_dram_parameter("bp2", [F, 1], dt.float32, isOutput=False)
    W1_p = nc.declare_dram_parameter("W1sb", [L, F, F], dt.bfloat16, isOutput=False)
    b1_p = nc.declare_dram_parameter("b1s", [L, F, 1], dt.float32, isOutput=False)
    W2_p = nc.declare_dram_parameter("W2sb", [L, F, F], dt.bfloat16, isOutput=False)
    b2_p = nc.declare_dram_parameter("b2s", [L, F, 1], dt.float32, isOutput=False)
    out_p = nc.declare_dram_parameter("out", [G, L * F], dt.float32, isOutput=True)

    # ---- internal DRAM ----
    ag_in = nc.dram_tensor("ag_in", [PERP, F], dt.bfloat16)
    tables = [
        nc.dram_tensor(f"table{l}", [TROWS, F], dt.bfloat16, addr_space="Shared")
        for l in range(L)
    ]

    with tile.TileContext(nc) as tc:
        with (
            tc.tile_pool(name="const", bufs=1) as cpool,
            tc.tile_pool(name="ht", bufs=1) as hpool,
            tc.tile_pool(name="gath", bufs=20) as gpool,
            tc.tile_pool(name="idxp", bufs=20) as ipool,
            tc.tile_pool(name="oh", bufs=12) as ohpool,
            tc.tile_pool(name="ohsp", bufs=4) as ohsp_pool,
            tc.tile_pool(name="zz", bufs=2) as zpool,
            tc.tile_pool(name="emit", bufs=4) as epool,
            tc.tile_pool(name="psag", bufs=4, space="PSUM") as psag,
            tc.tile_pool(name="psmlp", bufs=2, space="PSUM") as psmlp,
            tc.tile_pool(name="pstr", bufs=1, space="PSUM") as pstr,
            tc.tile_pool(name="pspool", bufs=1, space="PSUM") as pspool,
        ):
            # ---- load constants / weights ----
            iota2_sb = cpool.tile([128, 128, OHT], dt.bfloat16, tag="iota2")
            nc.sync.dma_start(iota2_sb[:].rearrange("p a b -> p (a b)"), iota2_p[:])
            iota512_sb = cpool.tile([128, 512], dt.float32, tag="iota512")
            nc.sync.dma_start(iota512_sb[:], iota512_p[:])
            ident_sb = cpool.tile([128, 128], dt.bfloat16, tag="ident")
            nc.sync.dma_start(ident_sb[:], ident_p[:])
            bg_sb = cpool.tile([128, NBLK], dt.bfloat16, tag="bg")
            nc.sync.dma_start(bg_sb[:], bg_p[:])
            drel_sb = cpool.tile([128, tot_tiles], dt.bfloat16, tag="drel")
            nc.sync.dma_start(drel_sb[:], drel_p[:])
            drsp_sb = cpool.tile([128, n_sp_tiles], dt.float32, tag="drsp")
            nc.sync.dma_start(drsp_sb[:], drsp_p[:])

            def _load_w(tag, pslice):
                t = cpool.tile([F, F], dt.bfloat16, tag=tag)
                nc.sync.dma_start(t[:], pslice)
                return t

            def _load_b(tag, pslice):
                t = cpool.tile([F, 1], dt.float32, tag=tag)
                nc.sync.dma_start(t[:], pslice)
                return t

            Wp1 = _load_w("Wp1", Wp1_p[:])
            Wp2 = _load_w("Wp2", Wp2_p[:])
            bp1 = _load_b("bp1", bp1_p[:])
            bp2 = _load_b("bp2", bp2_p[:])
            W1 = [_load_w(f"W1_{l}", W1_p[l][:]) for l in range(L)]
            W2 = [_load_w(f"W2_{l}", W2_p[l][:]) for l in range(L)]
            b1 = [_load_b(f"b1_{l}", b1_p[l][:]) for l in range(L)]
            b2 = [_load_b(f"b2_{l}", b2_p[l][:]) for l in range(L)]
            # eps-scaled identities: z-matmul lhsT, one per layer
            epsI = []
            for l in range(L):
                t = cpool.tile([128, 128], dt.bfloat16, tag=f"epsI{l}")
                nc.vector.tensor_scalar(t[:], ident_sb[:],
                                        float(1.0 + eps_vals[l]), None,
                                        op0=OP.mult)
                epsI.append(t)

            hT = hpool.tile([F, PERP], dt.bfloat16, tag="hT")

            # ---- pre-MLP: hT = relu(relu(x Wp1 + bp1) Wp2 + bp2), transposed ----
            MLP_CHUNK = 512
            o = 0
            while o < PERP:
                cw = min(MLP_CHUNK, PERP - o)
                xc = zpool.tile([F, MLP_CHUNK], dt.bfloat16, tag="xc")
                nc.sync.dma_start(xc[:, :cw], xT_p[:, o:o + cw])
                p1 = psmlp.tile([F, MLP_CHUNK], dt.float32, tag="mlp")
                nc.tensor.matmul(p1[:, :cw], Wp1[:], xc[:, :cw])
                t1 = zpool.tile([F, MLP_CHUNK], dt.bfloat16, tag="t1")
                nc.scalar.activation(t1[:, :cw], p1[:, :cw], AF.Relu, bias=bp1[:])
                p2 = psmlp.tile([F, MLP_CHUNK], dt.float32, tag="mlp")
                nc.tensor.matmul(p2[:, :cw], Wp2[:], t1[:, :cw])
                nc.scalar.activation(hT[:, o:o + cw], p2[:, :cw], AF.Relu, bias=bp2[:])
                o += cw

            pool_psums = []

            def _emit_block(b, layer_out):
                """Transpose block b of hT; DMA to ag_in (if a table is still
                needed) and accumulate pooling (if layer_out >= 1)."""
                ptr = pstr.tile([128, 128], dt.bfloat16, tag="tr")
                nc.tensor.transpose(ptr[:], hT[:, b * BLK:(b + 1) * BLK],
                                    ident_sb[:])
                hrow = epool.tile([128, 128], dt.bfloat16, tag="hrow")
                nc.scalar.activation(hrow[:], ptr[:], AF.Copy)
                if layer_out < L:
                    nc.sync.dma_start(ag_in[b * BLK:(b + 1) * BLK, :], hrow[:])
                if layer_out >= 1:
                    ohg = epool.tile([128, G], dt.bfloat16, tag="ohg")
                    nc.vector.tensor_tensor(
                        ohg[:],
                        bg_sb[:, b:b + 1].broadcast_to([128, G]),
                        iota2_sb[:, :G, 0],
                        OP.is_equal,
                    )
                    nc.tensor.matmul(
                        pool_psums[layer_out - 1][:],
                        ohg[:],
                        hrow[:],
                        start=(b == 0),
                        stop=(b == NBLK - 1),
                        skip_group_check=True,
                    )

            def _ag_chunk(l_out, lo, hi):
                """AllGather ag_in[lo:hi] into the matching contiguous table
                half (table layout is [cores x rows<HALF_P][cores x rest])."""
                base = 0 if lo == 0 else NC * HALF_P
                nc.gpsimd.collective_compute(
                    "AllGather", OP.bypass,
                    replica_groups=[list(range(NC))],
                    ins=[ag_in[lo:hi, :]],
                    outs=[tables[l_out][base:base + NC * (hi - lo), :]],
                )

            # emit h0 -> table0 (AllGather fired in 2 chunks to overlap)
            for b in range(NBLK):
                _emit_block(b, 0)
                if b == HALF_P // BLK - 1:
                    _ag_chunk(0, 0, HALF_P)
            _ag_chunk(0, HALF_P, PERP)

            # ---- GIN layers: 3-stage pipeline over rounds ----
            for l in range(L):
                pool_psums.append(pspool.tile([G, F], dt.float32, tag="pool",
                                              name=f"poolp{l}"))

                def _issue_gathers(r):
                    gts = {}
                    for (rr, q, call_off, n_slots) in call_meta:
                        if rr != r or n_slots == 0:
                            continue
                        idxs = ipool.tile([128, n_slots // 16], dt.int16,
                                          tag="idxs", name=f"idxs_l{l}_r{r}_q{q}")
                        nc.sync.dma_start(
                            idxs[:], idx_p[:, call_off // 16:(call_off + n_slots) // 16]
                        )
                        gt = gpool.tile([128, n_slots // BLK, 128], dt.bfloat16,
                                        tag="gt", name=f"gt_l{l}_r{r}_q{q}")
                        nc.gpsimd.dma_gather(
                            gt[:],
                            tables[l][q * QS:(q + 1) * QS, :],
                            idxs[:],
                            n_slots,
                            n_slots,
                            F,
                            single_packet=False,
                            queue_num=q,
                        )
                        gts[q] = (gt, call_off // BLK)
                    return gts

                def _aggregate(r, gts):
                    rblocks = [b for b in _round_blocks(r)]
                    cw = len(rblocks) * BLK
                    o = rblocks[0] * BLK
                    agg_r = psag.tile([F, RBLK * BLK], dt.float32, tag="agg",
                                      name=f"agg_l{l}_r{r}")
                    # start=True resets the whole PSUM bank, so windows can't
                    # each open their own group: zero once, all-accumulate.
                    nc.vector.memset(agg_r[:], 0.0)
                    for q in range(NQ):
                        t0 = cell_tile[(q, rblocks[0])]
                        Tr = KC * len(rblocks)
                        oh = ohpool.tile([128, 128, OHT], dt.bfloat16, tag="oh",
                                         name=f"oh_l{l}_r{r}_q{q}")
                        nc.vector.tensor_tensor(
                            oh[:, :, :Tr],
                            drel_sb[:, t0:t0 + Tr].unsqueeze(1)
                                .broadcast_to([128, 128, Tr]),
                            iota2_sb[:, :, :Tr],
                            OP.is_equal,
                        )
                        gt, c0 = gts[q]
                        for w, b in enumerate(rblocks):
                            bt = cell_tile[(q, b)]
                            for t in range(KC):
                                nc.tensor.matmul(
                                    agg_r[:, w * BLK:(w + 1) * BLK],
                                    gt[:, bt - c0 + t, :],
                                    oh[:, :, bt - t0 + t],
                                    start=False,
                                    stop=False,
                                    skip_group_check=True,
                                )
                        # spill matmuls: 512-wide one-hots over the full bank
                        ksp = int(Ksp[q, r])
                        st0 = spill_tile[(q, r)]
                        sp0 = sp_packed[(q, r)]
                        for t in range(ksp):
                            ohsp = ohsp_pool.tile([128, 512], dt.bfloat16,
                                                  tag="ohsp",
                                                  name=f"ohsp_l{l}_r{r}_q{q}_{t}")
                            nc.vector.tensor_tensor(
                                ohsp[:],
                                drsp_sb[:, sp0 + t:sp0 + t + 1]
                                    .broadcast_to([128, 512]),
                                iota512_sb[:],
                                OP.is_equal,
                            )
                            nc.tensor.matmul(
                                agg_r[:],
                                gt[:, st0 - c0 + t, :],
                                ohsp[:],
                                start=False,
                                stop=False,
                                skip_group_check=True,
                            )
                    # z = agg + (1+eps) h via eps-scaled identity matmul
                    nc.tensor.matmul(
                        agg_r[:, :cw],
                        epsI[l][:],
                        hT[:, o:o + cw],
                        start=False,
                        stop=True,
                        skip_group_check=True,
                    )
                    return (o, cw, agg_r)

                def _mlp_chunk(o, cw, agg_r):
                    # z was accumulated in PSUM: (1+eps)h + agg; copy to SBUF bf16
                    z = zpool.tile([F, MLP_CHUNK], dt.bfloat16, tag="z",
                                   name=f"z_l{l}_o{o}")
                    nc.scalar.activation(z[:, :cw], agg_r[:, :cw], AF.Copy)
                    p1 = psmlp.tile([F, MLP_CHUNK], dt.float32, tag="mlp",
                                    name=f"p1_l{l}_o{o}")
                    nc.tensor.matmul(p1[:, :cw], W1[l][:], z[:, :cw])
                    t1 = zpool.tile([F, MLP_CHUNK], dt.bfloat16, tag="t1",
                                    name=f"t1_l{l}_o{o}")
                    nc.scalar.activation(t1[:, :cw], p1[:, :cw], AF.Relu, bias=b1[l][:])
                    p2 = psmlp.tile([F, MLP_CHUNK], dt.float32, tag="mlp",
                                    name=f"p2_l{l}_o{o}")
                    nc.tensor.matmul(p2[:, :cw], W2[l][:], t1[:, :cw])
                    nc.scalar.activation(hT[:, o:o + cw], p2[:, :cw], AF.Identity,
                                         bias=b2[l][:])
                    for k in range(cw // BLK):
                        _emit_block(o // BLK + k, l + 1)

                # pipeline: gathers(r) | aggregate(r-1) | mlp+emit(r-2)
                gts_q = {}
                agg_q = {}
                ag1_round = (HALF_P // BLK) // RBLK  # mlp of this round
                for r in range(NR + 2):
                    if r < NR:
                        gts_q[r] = _issue_gathers(r)
                    if 0 <= r - 1 < NR:
                        agg_q[r - 1] = _aggregate(r - 1, gts_q.pop(r - 1))
                    if 0 <= r - 2:
                        _mlp_chunk(*agg_q.pop(r - 2))
                        if l + 1 < L and r - 2 == ag1_round:
                            _ag_chunk(l + 1, 0, HALF_P)

                if l + 1 < L:
                    _ag_chunk(l + 1, HALF_P, PERP)
                # extract pooled sums for this layer
                pooled_sb = epool.tile([G, F], dt.float32, tag="pooled")
                nc.scalar.activation(pooled_sb[:], pool_psums[l][:], AF.Copy)
                nc.sync.dma_start(out_p[:, l * F:(l + 1) * F], pooled_sb[:])

    nc.compile()
    return nc


def _make_in_maps(struct, inputs):
    x = np.asarray(inputs["x"], dtype=_F32)
    OHT = KC * RBLK
    iota2 = np.broadcast_to(
        np.arange(128, dtype=_F32)[:, None], (128, 128, OHT)
    ).reshape(128, 128 * OHT).astype(_BF16)  # iota2[p, j*OHT + t] = j
    iota512 = np.ascontiguousarray(np.broadcast_to(
        np.arange(512, dtype=_F32), (128, 512)))
    ident = np.eye(128, dtype=_F32).astype(_BF16)

    shared = {
        "iota2": np.ascontiguousarray(iota2),
        "iota512": iota512,
        "ident": np.ascontiguousarray(ident),
        "Wp1b": np.asarray(inputs["W_pre1"]).astype(_BF16),
        "bp1": np.asarray(inputs["b_pre1"], dtype=_F32).reshape(F, 1),
        "Wp2b": np.asarray(inputs["W_pre2"]).astype(_BF16),
        "bp2": np.asarray(inputs["b_pre2"], dtype=_F32).reshape(F, 1),
        "W1sb": np.asarray(inputs["W1s"]).astype(_BF16),
        "b1s": np.asarray(inputs["b1s"], dtype=_F32).reshape(L, F, 1),
        "W2sb": np.asarray(inputs["W2s"]).astype(_BF16),
        "b2s": np.asarray(inputs["b2s"], dtype=_F32).reshape(L, F, 1),
    }

    in_maps = []
    for c in range(NC):
        xs = np.zeros((F, PERP), dtype=_F32)
        xs[:, :PER] = x[c * PER:(c + 1) * PER].T
        m = dict(shared)
        m["xTb"] = xs.astype(_BF16)
        m["idx"] = struct["idx_planes"][c]
        m["dstrel"] = struct["drel_planes"][c]
        m["dstrelsp"] = struct["drsp_planes"][c]
        m["bg"] = struct["bg_planes"][c]
        in_maps.append(m)
    return in_maps


def kernel(**inputs):
    from concourse.bass_utils import run_bass_kernel_spmd

    edge_index = np.asarray(inputs["edge_index"])
    batch = np.asarray(inputs["batch"])
    eps = np.asarray(inputs["eps"], dtype=_F32)

    struct = _build_structures(edge_index, batch)
    nc = _build_program(struct, [float(e) for e in eps])
    in_maps = _make_in_maps(struct, inputs)

    res = run_bass_kernel_spmd(nc, in_maps, core_ids=list(range(NC)))
    out = np.zeros((G, L * F), dtype=_F32)
    for c in range(NC):
        out += res.results[c]["out"]
    return out


# revision 30
# speedup vs baseline: 1.0235x; 1.0206x over previous
"""Distributed GIN (3-layer) kernel for 8 TRN2 NeuronCores.

Sharding: nodes (and their incident in-edges) are partitioned across the 8
cores; each core keeps its node-feature shard resident in SBUF (transposed,
bf16).  Between layers the per-core shards are AllGathered into a bf16
"gather table" in DRAM; neighbor aggregation is a dma_gather (256B/edge) +
one-hot matmul segment-sum on the tensor engine.  MLP weights are replicated.
Per-graph pooled sums are computed per-core on the PE and reduced on the host.

Perf-critical structure:
- Every (quadrant, dst-block) cell gets exactly CAP=512 gather slots; overflow
  edges go to a per-(quadrant, round) spill stream aggregated with 512-wide
  one-hots.  This keeps dma_gather descriptor count (the Q7/SWDGE serial
  bottleneck) within ~7% of the true edge count.
- Gather calls are per (round, quadrant); the 4 quadrant calls run on the 4
  SWDGE queues, so their Q7 descriptor generation executes on 4 different
  core pairs concurrently (~4x the single-queue gather rate).
- 3-stage software pipeline per round r: gathers(r) / aggregation matmuls
  (r-1) / MLP+emits(r-2), with 4 rounds of gather buffering, so the PE is
  never gated on the ACT z-copy round trip and the gather stream free-runs.
- z = (1+eps)h + agg is accumulated on the PE (eps-scaled identity matmul);
  the whole MLP runs in bf16.
- The gather table is two stacked halves (all cores' rows < HALF_P, then the
  rest) so the per-layer AllGather fires as two contiguous chunks, the first
  mid-layer, overlapping the collective with compute.
"""

import math

import ml_dtypes
import numpy as np

N_NODES = 100000
N_EDGES = 1600000
F = 128              # feature dim (= hidden dim)
L = 3                # GIN layers
G = 64               # graphs
NC = 8               # NeuronCores
PER = N_NODES // NC  # 12500 real nodes per core
BLK = 128
NBLK = (PER + BLK - 1) // BLK          # 98 node blocks per core
PERP = NBLK * BLK                      # 12544 padded rows per core
TROWS = NC * PERP                      # 100352 table rows
NQ = 4
QS = TROWS // NQ                       # 25088 rows per quadrant (< int16 max)
RBLK = 4                               # dst blocks per PSUM round (1 full bank)
NR = (NBLK + RBLK - 1) // RBLK         # 25 rounds
CAP = 512                              # main slots per (q, b) cell (4 tiles)
KC = CAP // BLK                        # 4 tiles per cell
S1 = 49 * BLK                          # AllGather section 1 rows per core
S2 = 25 * BLK                          # section 2 (fires ~round 18)
S3 = PERP - S1 - S2                    # section 3 (small layer-end chunk)
HALF_P = S1                            # kept name: first-section boundary

_F32 = np.float32
_BF16 = ml_dtypes.bfloat16


def _table_row(node):
    """Global node id -> row in the (padded) all-gathered table.

    Table layout is two stacked halves so chunked AllGathers write
    contiguous slices: [all cores' rows 0:HALF_P][all cores' rows HALF_P:].
    """
    c = node // PER
    p = node % PER
    return np.where(
        p < S1, c * S1 + p,
        np.where(p < S1 + S2,
                 NC * S1 + c * S2 + (p - S1),
                 NC * (S1 + S2) + c * S3 + (p - S1 - S2)))


def _round_blocks(r):
    b0 = r * RBLK
    return range(b0, min(b0 + RBLK, NBLK))


def _build_structures(edge_index, batch):
    """Host-side preprocessing: per-core padded edge-slot streams.

    Layout per (r, q) gather call:
      [cell (q,b) : 512 slots for b in blocks(r)] ++
      [spill (q,r) : Ksp[q][r]*128 slots]
    """
    src = np.asarray(edge_index[0], dtype=np.int64)
    dst = np.asarray(edge_index[1], dtype=np.int64)
    srow = _table_row(src)
    q_of_edge = srow // QS
    qloc = (srow % QS).astype(np.int16)

    core_of_edge = dst // PER
    dst_local = dst % PER
    b_of_edge = dst_local // BLK

    # ---- per-core main/spill split (cap CAP per cell) ----
    per_core = []
    spill_counts = np.zeros((NC, NQ, NR), dtype=np.int64)
    for c in range(NC):
        sel = np.nonzero(core_of_edge == c)[0]
        eq = q_of_edge[sel]
        eb = b_of_edge[sel]
        order = np.lexsort((qloc[sel], eb, eq))
        sel = sel[order]
        eq, eb = eq[order], eb[order]
        gid = eq * NBLK + eb
        run_starts = np.zeros(NQ * NBLK + 1, dtype=np.int64)
        np.add.at(run_starts, gid + 1, 1)
        run_starts = np.cumsum(run_starts)
        rank = np.arange(gid.size) - run_starts[gid]
        is_spill = rank >= CAP
        er = eb // RBLK
        np.add.at(spill_counts, (np.full(sel.size, c)[is_spill],
                                 eq[is_spill], er[is_spill]), 1)
        per_core.append((sel, eq, eb, rank, is_spill))

    Ksp = np.ceil(spill_counts.max(axis=0) / BLK).astype(np.int64)  # [NQ, NR]

    # ---- static slot-stream layout: r-major, then q ----
    cell_tile = {}    # (q, b) -> global tile index of cell's first tile
    spill_tile = {}   # (q, r) -> global tile index of spill's first tile
    sp_packed = {}    # (q, r) -> packed spill-tile index (for the drsp plane)
    call_meta = []    # per (r, q): (slot_offset, n_slots)
    off = 0
    spk = 0
    for r in range(NR):
        for q in range(NQ):
            call_off = off
            for b in _round_blocks(r):
                cell_tile[(q, b)] = off // BLK
                off += CAP
            spill_tile[(q, r)] = off // BLK
            sp_packed[(q, r)] = spk
            off += int(Ksp[q, r]) * BLK
            spk += int(Ksp[q, r])
            call_meta.append((r, q, call_off, off - call_off))
    tot_slots = off
    tot_tiles = tot_slots // BLK
    n_sp_tiles = max(int(Ksp.sum()), 1)

    idx_planes = []
    drel_planes = []
    drsp_planes = []
    bg_planes = []
    flat_cell = np.zeros(NQ * NBLK, dtype=np.int64)
    for (q, b), t in cell_tile.items():
        flat_cell[q * NBLK + b] = t * BLK
    flat_sp = np.zeros(NQ * NR, dtype=np.int64)
    flat_sp_packed = np.zeros(NQ * NR, dtype=np.int64)
    for (q, r), t in spill_tile.items():
        flat_sp[q * NR + r] = t * BLK
        flat_sp_packed[q * NR + r] = sp_packed[(q, r)] * BLK
    for c in range(NC):
        sel, eq, eb, rank, is_spill = per_core[c]
        idx = np.zeros(tot_slots, dtype=np.int16)
        drel = np.full(tot_slots, -1.0, dtype=np.float32)
        drsp = np.full(n_sp_tiles * BLK, -1.0, dtype=np.float32)

        # main slots
        m = ~is_spill
        gid_m = eq[m] * NBLK + eb[m]
        pos_m = flat_cell[gid_m] + rank[m]
        idx[pos_m] = qloc[sel[m]]
        drel[pos_m] = (dst_local[sel[m]] - eb[m] * BLK).astype(np.float32)

        # spill slots: per (q, r) ordered by srow
        s = is_spill
        if s.any():
            eqs = eq[s]
            ers = eb[s] // RBLK
            srt = np.lexsort((qloc[sel[s]], ers, eqs))
            eqs, ers = eqs[srt], ers[srt]
            sel_s = sel[s][srt]
            gid_s = eqs * NR + ers
            rs = np.zeros(NQ * NR + 1, dtype=np.int64)
            np.add.at(rs, gid_s + 1, 1)
            rs = np.cumsum(rs)
            rank_s = np.arange(gid_s.size) - rs[gid_s]
            idx[flat_sp[gid_s] + rank_s] = qloc[sel_s]
            drsp[flat_sp_packed[gid_s] + rank_s] = (
                dst_local[sel_s] - ers * RBLK * BLK).astype(np.float32)

        idx_plane = np.tile(idx.reshape(-1, 16).T, (8, 1))          # [128, tot/16]
        drel_plane = drel.reshape(-1, BLK).T.astype(_BF16)          # [128, tot_tiles]
        drsp_plane = drsp.reshape(-1, BLK).T.astype(_F32)           # [128, n_sp_tiles]
        idx_planes.append(np.ascontiguousarray(idx_plane))
        drel_planes.append(np.ascontiguousarray(drel_plane))
        drsp_planes.append(np.ascontiguousarray(drsp_plane))

        # per-node graph ids (bf16 values; -1 pad), for on-the-fly pool one-hots
        bgfull = np.full(NBLK * BLK, -1.0, dtype=np.float32)
        bgfull[:PER] = np.asarray(batch[c * PER:(c + 1) * PER], dtype=np.float32)
        bg_planes.append(np.ascontiguousarray(
            bgfull.reshape(NBLK, BLK).T.astype(_BF16)))

    return {
        "Ksp": Ksp,
        "cell_tile": cell_tile,
        "spill_tile": spill_tile,
        "sp_packed": sp_packed,
        "call_meta": call_meta,
        "tot_slots": tot_slots,
        "tot_tiles": tot_tiles,
        "n_sp_tiles": n_sp_tiles,
        "idx_planes": idx_planes,
        "drel_planes": drel_planes,
        "drsp_planes": drsp_planes,
        "bg_planes": bg_planes,
    }


def _build_program(struct, eps_vals):
    import concourse.bacc as bacc
    import concourse.mybir as mybir
    from concourse import tile

    dt = mybir.dt
    AF = mybir.ActivationFunctionType
    OP = mybir.AluOpType

    tot_slots = struct["tot_slots"]
    tot_tiles = struct["tot_tiles"]
    n_sp_tiles = struct["n_sp_tiles"]
    call_meta = struct["call_meta"]
    cell_tile = struct["cell_tile"]
    spill_tile = struct["spill_tile"]
    sp_packed = struct["sp_packed"]
    Ksp = struct["Ksp"]
    OHT = KC * RBLK  # 16 main one-hot tiles per (q, round)

    nc = bacc.Bacc("TRN2", target_bir_lowering=False, num_swdge_queues=4)

    # ---- kernel parameters (per-core values via in_maps) ----
    xT_p = nc.declare_dram_parameter("xTb", [F, PERP], dt.bfloat16, isOutput=False)
    idx_p = nc.declare_dram_parameter("idx", [128, tot_slots // 16], dt.int16, isOutput=False)
    drel_p = nc.declare_dram_parameter("dstrel", [128, tot_tiles], dt.bfloat16, isOutput=False)
    drsp_p = nc.declare_dram_parameter("dstrelsp", [128, n_sp_tiles], dt.float32, isOutput=False)
    bg_p = nc.declare_dram_parameter("bg", [128, NBLK], dt.bfloat16, isOutput=False)
    iota2_p = nc.declare_dram_parameter("iota2", [128, 128 * OHT], dt.bfloat16, isOutput=False)
    iota512_p = nc.declare_dram_parameter("iota512", [128, 512], dt.float32, isOutput=False)
    ident_p = nc.declare_dram_parameter("ident", [128, 128], dt.bfloat16, isOutput=False)
    Wp1_p = nc.declare_dram_parameter("Wp1b", [F, F], dt.bfloat16, isOutput=False)
    bp1_p = nc.declare_dram_parameter("bp1", [F, 1], dt.float32, isOutput=False)
    Wp2_p = nc.declare_dram_parameter("Wp2b", [F, F], dt.bfloat16, isOutput=False)
    bp2_p = nc.declare_dram_parameter("bp2", [F, 1], dt.float32, isOutput=False)
    W1_p = nc.declare_dram_parameter("W1sb", [L, F, F], dt.bfloat16, isOutput=False)
    b1_p = nc.declare_dram_parameter("b1s", [L, F, 1], dt.float32, isOutput=False)
    W2_p = nc.declare_dram_parameter("W2sb", [L, F, F], dt.bfloat16, isOutput=False)
    b2_p = nc.declare_dram_parameter("b2s", [L, F, 1], dt.float32, isOutput=False)
    out_p = nc.declare_dram_parameter("out", [G, L * F], dt.float32, isOutput=True)

    # ---- internal DRAM ----
    ag_in = nc.dram_tensor("ag_in", [PERP, F], dt.bfloat16)
    tables = [
        nc.dram_tensor(f"table{l}", [TROWS, F], dt.bfloat16, addr_space="Shared")
        for l in range(L)
    ]

    with tile.TileContext(nc) as tc:
        with (
            tc.tile_pool(name="const", bufs=1) as cpool,
            tc.tile_pool(name="ht", bufs=1) as hpool,
            tc.tile_pool(name="gath", bufs=20) as gpool,
            tc.tile_pool(name="idxp", bufs=20) as ipool,
            tc.tile_pool(name="oh", bufs=12) as ohpool,
            tc.tile_pool(name="ohsp", bufs=4) as ohsp_pool,
            tc.tile_pool(name="zz", bufs=2) as zpool,
            tc.tile_pool(name="emit", bufs=4) as epool,
            tc.tile_pool(name="psag", bufs=4, space="PSUM") as psag,
            tc.tile_pool(name="psmlp", bufs=2, space="PSUM") as psmlp,
            tc.tile_pool(name="pstr", bufs=1, space="PSUM") as pstr,
            tc.tile_pool(name="pspool", bufs=1, space="PSUM") as pspool,
        ):
            # ---- load constants / weights ----
            iota2_sb = cpool.tile([128, 128, OHT], dt.bfloat16, tag="iota2")
            nc.sync.dma_start(iota2_sb[:].rearrange("p a b -> p (a b)"), iota2_p[:])
            iota512_sb = cpool.tile([128, 512], dt.float32, tag="iota512")
            nc.sync.dma_start(iota512_sb[:], iota512_p[:])
            ident_sb = cpool.tile([128, 128], dt.bfloat16, tag="ident")
            nc.sync.dma_start(ident_sb[:], ident_p[:])
            bg_sb = cpool.tile([128, NBLK], dt.bfloat16, tag="bg")
            nc.sync.dma_start(bg_sb[:], bg_p[:])
            drel_sb = cpool.tile([128, tot_tiles], dt.bfloat16, tag="drel")
            nc.sync.dma_start(drel_sb[:], drel_p[:])
            drsp_sb = cpool.tile([128, n_sp_tiles], dt.float32, tag="drsp")
            nc.sync.dma_start(drsp_sb[:], drsp_p[:])

            def _load_w(tag, pslice):
                t = cpool.tile([F, F], dt.bfloat16, tag=tag)
                nc.sync.dma_start(t[:], pslice)
                return t

            def _load_b(tag, pslice):
                t = cpool.tile([F, 1], dt.float32, tag=tag)
                nc.sync.dma_start(t[:], pslice)
                return t

            Wp1 = _load_w("Wp1", Wp1_p[:])
            Wp2 = _load_w("Wp2", Wp2_p[:])
            bp1 = _load_b("bp1", bp1_p[:])
            bp2 = _load_b("bp2", bp2_p[:])
            W1 = [_load_w(f"W1_{l}", W1_p[l][:]) for l in range(L)]
            W2 = [_load_w(f"W2_{l}", W2_p[l][:]) for l in range(L)]
            b1 = [_load_b(f"b1_{l}", b1_p[l][:]) for l in range(L)]
            b2 = [_load_b(f"b2_{l}", b2_p[l][:]) for l in range(L)]
            # eps-scaled identities: z-matmul lhsT, one per layer
            epsI = []
            for l in range(L):
                t = cpool.tile([128, 128], dt.bfloat16, tag=f"epsI{l}")
                nc.vector.tensor_scalar(t[:], ident_sb[:],
                                        float(1.0 + eps_vals[l]), None,
                                        op0=OP.mult)
                epsI.append(t)

            hT = hpool.tile([F, PERP], dt.bfloat16, tag="hT")

            # ---- pre-MLP: hT = relu(relu(x Wp1 + bp1) Wp2 + bp2), transposed ----
            MLP_CHUNK = 512
            o = 0
            while o < PERP:
                cw = min(MLP_CHUNK, PERP - o)
                xc = zpool.tile([F, MLP_CHUNK], dt.bfloat16, tag="xc")
                nc.sync.dma_start(xc[:, :cw], xT_p[:, o:o + cw])
                p1 = psmlp.tile([F, MLP_CHUNK], dt.float32, tag="mlp")
                nc.tensor.matmul(p1[:, :cw], Wp1[:], xc[:, :cw])
                t1 = zpool.tile([F, MLP_CHUNK], dt.bfloat16, tag="t1")
                nc.scalar.activation(t1[:, :cw], p1[:, :cw], AF.Relu, bias=bp1[:])
                p2 = psmlp.tile([F, MLP_CHUNK], dt.float32, tag="mlp")
                nc.tensor.matmul(p2[:, :cw], Wp2[:], t1[:, :cw])
                nc.scalar.activation(hT[:, o:o + cw], p2[:, :cw], AF.Relu, bias=bp2[:])
                o += cw

            pool_psums = []

            def _emit_block(b, layer_out):
                """Transpose block b of hT; DMA to ag_in (if a table is still
                needed) and accumulate pooling (if layer_out >= 1)."""
                ptr = pstr.tile([128, 128], dt.bfloat16, tag="tr")
                nc.tensor.transpose(ptr[:], hT[:, b * BLK:(b + 1) * BLK],
                                    ident_sb[:])
                hrow = epool.tile([128, 128], dt.bfloat16, tag="hrow")
                nc.scalar.activation(hrow[:], ptr[:], AF.Copy)
                if layer_out < L:
                    nc.sync.dma_start(ag_in[b * BLK:(b + 1) * BLK, :], hrow[:])
                if layer_out >= 1:
                    ohg = epool.tile([128, G], dt.bfloat16, tag="ohg")
                    nc.vector.tensor_tensor(
                        ohg[:],
                        bg_sb[:, b:b + 1].broadcast_to([128, G]),
                        iota2_sb[:, :G, 0],
                        OP.is_equal,
                    )
                    nc.tensor.matmul(
                        pool_psums[layer_out - 1][:],
                        ohg[:],
                        hrow[:],
                        start=(b == 0),
                        stop=(b == NBLK - 1),
                        skip_group_check=True,
                    )

            def _ag_chunk(l_out, lo, hi):
                """AllGather ag_in[lo:hi] into the matching contiguous table
                section (table layout is three stacked per-core sections)."""
                base = {0: 0, S1: NC * S1, S1 + S2: NC * (S1 + S2)}[lo]
                nc.gpsimd.collective_compute(
                    "AllGather", OP.bypass,
                    replica_groups=[list(range(NC))],
                    ins=[ag_in[lo:hi, :]],
                    outs=[tables[l_out][base:base + NC * (hi - lo), :]],
                )

            # emit h0 -> table0 (AllGather fired in 3 chunks to overlap)
            for b in range(NBLK):
                _emit_block(b, 0)
                if b == S1 // BLK - 1:
                    _ag_chunk(0, 0, S1)
                elif b == (S1 + S2) // BLK - 1:
                    _ag_chunk(0, S1, S1 + S2)
            _ag_chunk(0, S1 + S2, PERP)

            # ---- GIN layers: 3-stage pipeline over rounds ----
            for l in range(L):
                pool_psums.append(pspool.tile([G, F], dt.float32, tag="pool",
                                              name=f"poolp{l}"))

                def _issue_gathers(r):
                    gts = {}
                    for (rr, q, call_off, n_slots) in call_meta:
                        if rr != r or n_slots == 0:
                            continue
                        idxs = ipool.tile([128, n_slots // 16], dt.int16,
                                          tag="idxs", name=f"idxs_l{l}_r{r}_q{q}")
                        nc.sync.dma_start(
                            idxs[:], idx_p[:, call_off // 16:(call_off + n_slots) // 16]
                        )
                        gt = gpool.tile([128, n_slots // BLK, 128], dt.bfloat16,
                                        tag="gt", name=f"gt_l{l}_r{r}_q{q}")
                        nc.gpsimd.dma_gather(
                            gt[:],
                            tables[l][q * QS:(q + 1) * QS, :],
                            idxs[:],
                            n_slots,
                            n_slots,
                            F,
                            single_packet=False,
                            queue_num=q,
                        )
                        gts[q] = (gt, call_off // BLK)
                    return gts

                def _aggregate(r, gts):
                    rblocks = [b for b in _round_blocks(r)]
                    cw = len(rblocks) * BLK
                    o = rblocks[0] * BLK
                    agg_r = psag.tile([F, RBLK * BLK], dt.float32, tag="agg",
                                      name=f"agg_l{l}_r{r}")
                    # start=True resets the whole PSUM bank, so windows can't
                    # each open their own group: zero once, all-accumulate.
                    nc.vector.memset(agg_r[:], 0.0)
                    for q in range(NQ):
                        t0 = cell_tile[(q, rblocks[0])]
                        Tr = KC * len(rblocks)
                        oh = ohpool.tile([128, 128, OHT], dt.bfloat16, tag="oh",
                                         name=f"oh_l{l}_r{r}_q{q}")
                        nc.vector.tensor_tensor(
                            oh[:, :, :Tr],
                            drel_sb[:, t0:t0 + Tr].unsqueeze(1)
                                .broadcast_to([128, 128, Tr]),
                            iota2_sb[:, :, :Tr],
                            OP.is_equal,
                        )
                        gt, c0 = gts[q]
                        for w, b in enumerate(rblocks):
                            bt = cell_tile[(q, b)]
                            for t in range(KC):
                                nc.tensor.matmul(
                                    agg_r[:, w * BLK:(w + 1) * BLK],
                                    gt[:, bt - c0 + t, :],
                                    oh[:, :, bt - t0 + t],
                                    start=False,
                                    stop=False,
                                    skip_group_check=True,
                                )
                        # spill matmuls: 512-wide one-hots over the full bank
                        ksp = int(Ksp[q, r])
                        st0 = spill_tile[(q, r)]
                        sp0 = sp_packed[(q, r)]
                        for t in range(ksp):
                            ohsp = ohsp_pool.tile([128, 512], dt.bfloat16,
                                                  tag="ohsp",
                                                  name=f"ohsp_l{l}_r{r}_q{q}_{t}")
                            nc.vector.tensor_tensor(
                                ohsp[:],
                                drsp_sb[:, sp0 + t:sp0 + t + 1]
                                    .broadcast_to([128, 512]),
                                iota512_sb[:],
                                OP.is_equal,
                            )
                            nc.tensor.matmul(
                                agg_r[:],
                                gt[:, st0 - c0 + t, :],
                                ohsp[:],
                                start=False,
                                stop=False,
                                skip_group_check=True,
                            )
                    # z = agg + (1+eps) h via eps-scaled identity matmul
                    nc.tensor.matmul(
                        agg_r[:, :cw],
                        epsI[l][:],
                        hT[:, o:o + cw],
                        start=False,
                        stop=True,
                        skip_group_check=True,
                    )
                    return (o, cw, agg_r)

                def _mlp_chunk(o, cw, agg_r):
                    # z was accumulated in PSUM: (1+eps)h + agg; copy to SBUF bf16
                    z = zpool.tile([F, MLP_CHUNK], dt.bfloat16, tag="z",
                                   name=f"z_l{l}_o{o}")
                    nc.scalar.activation(z[:, :cw], agg_r[:, :cw], AF.Copy)
                    p1 = psmlp.tile([F, MLP_CHUNK], dt.float32, tag="mlp",
                                    name=f"p1_l{l}_o{o}")
                    nc.tensor.matmul(p1[:, :cw], W1[l][:], z[:, :cw])
                    t1 = zpool.tile([F, MLP_CHUNK], dt.bfloat16, tag="t1",
                                    name=f"t1_l{l}_o{o}")
                    nc.scalar.activation(t1[:, :cw], p1[:, :cw], AF.Relu, bias=b1[l][:])
                    p2 = psmlp.tile([F, MLP_CHUNK], dt.float32, tag="mlp",
                                    name=f"p2_l{l}_o{o}")
                    nc.tensor.matmul(p2[:, :cw], W2[l][:], t1[:, :cw])
                    nc.scalar.activation(hT[:, o:o + cw], p2[:, :cw], AF.Identity,
                                         bias=b2[l][:])
                    for k in range(cw // BLK):
                        _emit_block(o // BLK + k, l + 1)

                # pipeline: gathers(r) | aggregate(r-1) | mlp+emit(r-2)
                gts_q = {}
                agg_q = {}
                ag1_round = (S1 // BLK) // RBLK        # 12: blocks 0-48 done
                ag2_round = ((S1 + S2) // BLK) // RBLK  # 18: blocks 49-73 done
                for r in range(NR + 2):
                    if r < NR:
                        gts_q[r] = _issue_gathers(r)
                    if 0 <= r - 1 < NR:
                        agg_q[r - 1] = _aggregate(r - 1, gts_q.pop(r - 1))
                    if 0 <= r - 2:
                        _mlp_chunk(*agg_q.pop(r - 2))
                        if l + 1 < L and r - 2 == ag1_round:
                            _ag_chunk(l + 1, 0, S1)
                        elif l + 1 < L and r - 2 == ag2_round:
                            _ag_chunk(l + 1, S1, S1 + S2)

                if l + 1 < L:
                    _ag_chunk(l + 1, S1 + S2, PERP)
                # extract pooled sums for this layer
                pooled_sb = epool.tile([G, F], dt.float32, tag="pooled")
                nc.scalar.activation(pooled_sb[:], pool_psums[l][:], AF.Copy)
                nc.sync.dma_start(out_p[:, l * F:(l + 1) * F], pooled_sb[:])

    nc.compile()
    return nc


def _make_in_maps(struct, inputs):
    x = np.asarray(inputs["x"], dtype=_F32)
    OHT = KC * RBLK
    iota2 = np.broadcast_to(
        np.arange(128, dtype=_F32)[:, None], (128, 128, OHT)
    ).reshape(128, 128 * OHT).astype(_BF16)  # iota2[p, j*OHT + t] = j
    iota512 = np.ascontiguousarray(np.broadcast_to(
        np.arange(512, dtype=_F32), (128, 512)))
    ident = np.eye(128, dtype=_F32).astype(_BF16)

    shared = {
        "iota2": np.ascontiguousarray(iota2),
        "iota512": iota512,
        "ident": np.ascontiguousarray(ident),
        "Wp1b": np.asarray(inputs["W_pre1"]).astype(_BF16),
        "bp1": np.asarray(inputs["b_pre1"], dtype=_F32).reshape(F, 1),
        "Wp2b": np.asarray(inputs["W_pre2"]).astype(_BF16),
        "bp2": np.asarray(inputs["b_pre2"], dtype=_F32).reshape(F, 1),
        "W1sb": np.asarray(inputs["W1s"]).astype(_BF16),
        "b1s": np.asarray(inputs["b1s"], dtype=_F32).reshape(L, F, 1),
        "W2sb": np.asarray(inputs["W2s"]).astype(_BF16),
        "b2s": np.asarray(inputs["b2s"], dtype=_F32).reshape(L, F, 1),
    }

    in_maps = []
    for c in range(NC):
        xs = np.zeros((F, PERP), dtype=_F32)
        xs[:, :PER] = x[c * PER:(c + 1) * PER].T
        m = dict(shared)
        m["xTb"] = xs.astype(_BF16)
        m["idx"] = struct["idx_planes"][c]
        m["dstrel"] = struct["drel_planes"][c]
        m["dstrelsp"] = struct["drsp_planes"][c]
        m["bg"] = struct["bg_planes"][c]
        in_maps.append(m)
    return in_maps


def kernel(**inputs):
    from concourse.bass_utils import run_bass_kernel_spmd

    edge_index = np.asarray(inputs["edge_index"])
    batch = np.asarray(inputs["batch"])
    eps = np.asarray(inputs["eps"], dtype=_F32)

    struct = _build_structures(edge_index, batch)
    nc = _build_program(struct, [float(e) for e in eps])
    in_maps = _make_in_maps(struct, inputs)

    res = run_bass_kernel_spmd(nc, in_maps, core_ids=list(range(NC)))
    out = np.zeros((G, L * F), dtype=_F32)
    for c in range(NC):
        out += res.results[c]["out"]
    return out


# revision 32
# speedup vs baseline: 1.0261x; 1.0026x over previous
"""Distributed GIN (3-layer) kernel for 8 TRN2 NeuronCores.

Sharding: nodes (and their incident in-edges) are partitioned across the 8
cores; each core keeps its node-feature shard resident in SBUF (transposed,
bf16).  Between layers the per-core shards are AllGathered into a bf16
"gather table" in DRAM; neighbor aggregation is a dma_gather (256B/edge) +
one-hot matmul segment-sum on the tensor engine.  MLP weights are replicated.
Per-graph pooled sums are computed per-core on the PE and reduced on the host.

Perf-critical structure:
- Every (quadrant, dst-block) cell gets exactly CAP=512 gather slots; overflow
  edges go to a per-(quadrant, round) spill stream aggregated with 512-wide
  one-hots.  This keeps dma_gather descriptor count (the Q7/SWDGE serial
  bottleneck) within ~7% of the true edge count.
- Gather calls are per (round, quadrant); the 4 quadrant calls run on the 4
  SWDGE queues, so their Q7 descriptor generation executes on 4 different
  core pairs concurrently (~4x the single-queue gather rate).
- 3-stage software pipeline per round r: gathers(r) / aggregation matmuls
  (r-1) / MLP+emits(r-2), with 4 rounds of gather buffering, so the PE is
  never gated on the ACT z-copy round trip and the gather stream free-runs.
- z = (1+eps)h + agg is accumulated on the PE (eps-scaled identity matmul);
  the whole MLP runs in bf16.
- The gather table is two stacked halves (all cores' rows < HALF_P, then the
  rest) so the per-layer AllGather fires as two contiguous chunks, the first
  mid-layer, overlapping the collective with compute.
"""

import math

import ml_dtypes
import numpy as np

N_NODES = 100000
N_EDGES = 1600000
F = 128              # feature dim (= hidden dim)
L = 3                # GIN layers
G = 64               # graphs
NC = 8               # NeuronCores
PER = N_NODES // NC  # 12500 real nodes per core
BLK = 128
NBLK = (PER + BLK - 1) // BLK          # 98 node blocks per core
PERP = NBLK * BLK                      # 12544 padded rows per core
TROWS = NC * PERP                      # 100352 table rows
NQ = 4
QS = TROWS // NQ                       # 25088 rows per quadrant (< int16 max)
RBLK = 4                               # dst blocks per PSUM round (1 full bank)
NR = (NBLK + RBLK - 1) // RBLK         # 25 rounds
CAP = 512                              # main slots per (q, b) cell (4 tiles)
KC = CAP // BLK                        # 4 tiles per cell
S1 = 49 * BLK                          # AllGather section 1 rows per core
S2 = 25 * BLK                          # section 2 (fires ~round 18)
S3 = PERP - S1 - S2                    # section 3 (small layer-end chunk)
HALF_P = S1                            # kept name: first-section boundary

_F32 = np.float32
_BF16 = ml_dtypes.bfloat16


def _table_row(node):
    """Global node id -> row in the (padded) all-gathered table.

    Table layout is two stacked halves so chunked AllGathers write
    contiguous slices: [all cores' rows 0:HALF_P][all cores' rows HALF_P:].
    """
    c = node // PER
    p = node % PER
    return np.where(
        p < S1, c * S1 + p,
        np.where(p < S1 + S2,
                 NC * S1 + c * S2 + (p - S1),
                 NC * (S1 + S2) + c * S3 + (p - S1 - S2)))


def _round_blocks(r):
    b0 = r * RBLK
    return range(b0, min(b0 + RBLK, NBLK))


def _build_structures(edge_index, batch):
    """Host-side preprocessing: per-core padded edge-slot streams.

    Layout per (r, q) gather call:
      [cell (q,b) : 512 slots for b in blocks(r)] ++
      [spill (q,r) : Ksp[q][r]*128 slots]
    """
    src = np.asarray(edge_index[0], dtype=np.int64)
    dst = np.asarray(edge_index[1], dtype=np.int64)
    srow = _table_row(src)
    q_of_edge = srow // QS
    qloc = (srow % QS).astype(np.int16)

    core_of_edge = dst // PER
    dst_local = dst % PER
    b_of_edge = dst_local // BLK

    # ---- per-core main/spill split (cap CAP per cell) ----
    per_core = []
    spill_counts = np.zeros((NC, NQ, NR), dtype=np.int64)
    for c in range(NC):
        sel = np.nonzero(core_of_edge == c)[0]
        eq = q_of_edge[sel]
        eb = b_of_edge[sel]
        order = np.lexsort((qloc[sel], eb, eq))
        sel = sel[order]
        eq, eb = eq[order], eb[order]
        gid = eq * NBLK + eb
        run_starts = np.zeros(NQ * NBLK + 1, dtype=np.int64)
        np.add.at(run_starts, gid + 1, 1)
        run_starts = np.cumsum(run_starts)
        rank = np.arange(gid.size) - run_starts[gid]
        is_spill = rank >= CAP
        er = eb // RBLK
        np.add.at(spill_counts, (np.full(sel.size, c)[is_spill],
                                 eq[is_spill], er[is_spill]), 1)
        per_core.append((sel, eq, eb, rank, is_spill))

    Ksp = np.ceil(spill_counts.max(axis=0) / BLK).astype(np.int64)  # [NQ, NR]

    # ---- static slot-stream layout: r-major, then q ----
    cell_tile = {}    # (q, b) -> global tile index of cell's first tile
    spill_tile = {}   # (q, r) -> global tile index of spill's first tile
    sp_packed = {}    # (q, r) -> packed spill-tile index (for the drsp plane)
    call_meta = []    # per (r, q): (slot_offset, n_slots)
    off = 0
    spk = 0
    for r in range(NR):
        for q in range(NQ):
            call_off = off
            for b in _round_blocks(r):
                cell_tile[(q, b)] = off // BLK
                off += CAP
            spill_tile[(q, r)] = off // BLK
            sp_packed[(q, r)] = spk
            off += int(Ksp[q, r]) * BLK
            spk += int(Ksp[q, r])
            call_meta.append((r, q, call_off, off - call_off))
    tot_slots = off
    tot_tiles = tot_slots // BLK
    n_sp_tiles = max(int(Ksp.sum()), 1)

    idx_planes = []
    drel_planes = []
    drsp_planes = []
    bg_planes = []
    flat_cell = np.zeros(NQ * NBLK, dtype=np.int64)
    for (q, b), t in cell_tile.items():
        flat_cell[q * NBLK + b] = t * BLK
    flat_sp = np.zeros(NQ * NR, dtype=np.int64)
    flat_sp_packed = np.zeros(NQ * NR, dtype=np.int64)
    for (q, r), t in spill_tile.items():
        flat_sp[q * NR + r] = t * BLK
        flat_sp_packed[q * NR + r] = sp_packed[(q, r)] * BLK
    for c in range(NC):
        sel, eq, eb, rank, is_spill = per_core[c]
        idx = np.zeros(tot_slots, dtype=np.int16)
        drel = np.full(tot_slots, -1.0, dtype=np.float32)
        drsp = np.full(n_sp_tiles * BLK, -1.0, dtype=np.float32)

        # main slots
        m = ~is_spill
        gid_m = eq[m] * NBLK + eb[m]
        pos_m = flat_cell[gid_m] + rank[m]
        idx[pos_m] = qloc[sel[m]]
        drel[pos_m] = (dst_local[sel[m]] - eb[m] * BLK).astype(np.float32)

        # spill slots: per (q, r) ordered by srow
        s = is_spill
        if s.any():
            eqs = eq[s]
            ers = eb[s] // RBLK
            srt = np.lexsort((qloc[sel[s]], ers, eqs))
            eqs, ers = eqs[srt], ers[srt]
            sel_s = sel[s][srt]
            gid_s = eqs * NR + ers
            rs = np.zeros(NQ * NR + 1, dtype=np.int64)
            np.add.at(rs, gid_s + 1, 1)
            rs = np.cumsum(rs)
            rank_s = np.arange(gid_s.size) - rs[gid_s]
            idx[flat_sp[gid_s] + rank_s] = qloc[sel_s]
            drsp[flat_sp_packed[gid_s] + rank_s] = (
                dst_local[sel_s] - ers * RBLK * BLK).astype(np.float32)

        idx_plane = np.tile(idx.reshape(-1, 16).T, (8, 1))          # [128, tot/16]
        drel_plane = drel.reshape(-1, BLK).T.astype(_BF16)          # [128, tot_tiles]
        drsp_plane = drsp.reshape(-1, BLK).T.astype(_F32)           # [128, n_sp_tiles]
        idx_planes.append(np.ascontiguousarray(idx_plane))
        drel_planes.append(np.ascontiguousarray(drel_plane))
        drsp_planes.append(np.ascontiguousarray(drsp_plane))

        # per-node graph ids (bf16 values; -1 pad), for on-the-fly pool one-hots
        bgfull = np.full(NBLK * BLK, -1.0, dtype=np.float32)
        bgfull[:PER] = np.asarray(batch[c * PER:(c + 1) * PER], dtype=np.float32)
        bg_planes.append(np.ascontiguousarray(
            bgfull.reshape(NBLK, BLK).T.astype(_BF16)))

    return {
        "Ksp": Ksp,
        "cell_tile": cell_tile,
        "spill_tile": spill_tile,
        "sp_packed": sp_packed,
        "call_meta": call_meta,
        "tot_slots": tot_slots,
        "tot_tiles": tot_tiles,
        "n_sp_tiles": n_sp_tiles,
        "idx_planes": idx_planes,
        "drel_planes": drel_planes,
        "drsp_planes": drsp_planes,
        "bg_planes": bg_planes,
    }


def _build_program(struct, eps_vals):
    import concourse.bacc as bacc
    import concourse.mybir as mybir
    from concourse import tile

    dt = mybir.dt
    AF = mybir.ActivationFunctionType
    OP = mybir.AluOpType

    tot_slots = struct["tot_slots"]
    tot_tiles = struct["tot_tiles"]
    n_sp_tiles = struct["n_sp_tiles"]
    call_meta = struct["call_meta"]
    cell_tile = struct["cell_tile"]
    spill_tile = struct["spill_tile"]
    sp_packed = struct["sp_packed"]
    Ksp = struct["Ksp"]
    OHT = KC * RBLK  # 16 main one-hot tiles per (q, round)

    nc = bacc.Bacc("TRN2", target_bir_lowering=False, num_swdge_queues=4)

    # ---- kernel parameters (per-core values via in_maps) ----
    xT_p = nc.declare_dram_parameter("xTb", [F, PERP], dt.bfloat16, isOutput=False)
    idx_p = nc.declare_dram_parameter("idx", [128, tot_slots // 16], dt.int16, isOutput=False)
    drel_p = nc.declare_dram_parameter("dstrel", [128, tot_tiles], dt.bfloat16, isOutput=False)
    drsp_p = nc.declare_dram_parameter("dstrelsp", [128, n_sp_tiles], dt.float32, isOutput=False)
    bg_p = nc.declare_dram_parameter("bg", [128, NBLK], dt.bfloat16, isOutput=False)
    iota2_p = nc.declare_dram_parameter("iota2", [128, 128 * OHT], dt.bfloat16, isOutput=False)
    iota512_p = nc.declare_dram_parameter("iota512", [128, 512], dt.float32, isOutput=False)
    ident_p = nc.declare_dram_parameter("ident", [128, 128], dt.bfloat16, isOutput=False)
    Wp1_p = nc.declare_dram_parameter("Wp1b", [F, F], dt.bfloat16, isOutput=False)
    bp1_p = nc.declare_dram_parameter("bp1", [F, 1], dt.float32, isOutput=False)
    Wp2_p = nc.declare_dram_parameter("Wp2b", [F, F], dt.bfloat16, isOutput=False)
    bp2_p = nc.declare_dram_parameter("bp2", [F, 1], dt.float32, isOutput=False)
    W1_p = nc.declare_dram_parameter("W1sb", [L, F, F], dt.bfloat16, isOutput=False)
    b1_p = nc.declare_dram_parameter("b1s", [L, F, 1], dt.float32, isOutput=False)
    W2_p = nc.declare_dram_parameter("W2sb", [L, F, F], dt.bfloat16, isOutput=False)
    b2_p = nc.declare_dram_parameter("b2s", [L, F, 1], dt.float32, isOutput=False)
    out_p = nc.declare_dram_parameter("out", [G, L * F], dt.float32, isOutput=True)

    # ---- internal DRAM ----
    ag_in = nc.dram_tensor("ag_in", [PERP, F], dt.bfloat16)
    tables = [
        nc.dram_tensor(f"table{l}", [TROWS, F], dt.bfloat16, addr_space="Shared")
        for l in range(L)
    ]

    with tile.TileContext(nc) as tc:
        with (
            tc.tile_pool(name="const", bufs=1) as cpool,
            tc.tile_pool(name="ht", bufs=1) as hpool,
            tc.tile_pool(name="gath", bufs=20) as gpool,
            tc.tile_pool(name="idxp", bufs=20) as ipool,
            tc.tile_pool(name="oh", bufs=12) as ohpool,
            tc.tile_pool(name="ohsp", bufs=4) as ohsp_pool,
            tc.tile_pool(name="zz", bufs=2) as zpool,
            tc.tile_pool(name="emit", bufs=4) as epool,
            tc.tile_pool(name="psag", bufs=4, space="PSUM") as psag,
            tc.tile_pool(name="psmlp", bufs=2, space="PSUM") as psmlp,
            tc.tile_pool(name="pstr", bufs=1, space="PSUM") as pstr,
            tc.tile_pool(name="pspool", bufs=1, space="PSUM") as pspool,
        ):
            # ---- load constants / weights ----
            iota2_sb = cpool.tile([128, 128, OHT], dt.bfloat16, tag="iota2")
            nc.sync.dma_start(iota2_sb[:].rearrange("p a b -> p (a b)"), iota2_p[:])
            iota512_sb = cpool.tile([128, 512], dt.float32, tag="iota512")
            nc.sync.dma_start(iota512_sb[:], iota512_p[:])
            ident_sb = cpool.tile([128, 128], dt.bfloat16, tag="ident")
            nc.sync.dma_start(ident_sb[:], ident_p[:])
            bg_sb = cpool.tile([128, NBLK], dt.bfloat16, tag="bg")
            nc.sync.dma_start(bg_sb[:], bg_p[:])
            drel_sb = cpool.tile([128, tot_tiles], dt.bfloat16, tag="drel")
            nc.sync.dma_start(drel_sb[:], drel_p[:])
            drsp_sb = cpool.tile([128, n_sp_tiles], dt.float32, tag="drsp")
            nc.sync.dma_start(drsp_sb[:], drsp_p[:])

            def _load_w(tag, pslice):
                t = cpool.tile([F, F], dt.bfloat16, tag=tag)
                nc.sync.dma_start(t[:], pslice)
                return t

            def _load_b(tag, pslice):
                t = cpool.tile([F, 1], dt.float32, tag=tag)
                nc.sync.dma_start(t[:], pslice)
                return t

            Wp1 = _load_w("Wp1", Wp1_p[:])
            Wp2 = _load_w("Wp2", Wp2_p[:])
            bp1 = _load_b("bp1", bp1_p[:])
            bp2 = _load_b("bp2", bp2_p[:])
            W1 = [_load_w(f"W1_{l}", W1_p[l][:]) for l in range(L)]
            W2 = [_load_w(f"W2_{l}", W2_p[l][:]) for l in range(L)]
            b1 = [_load_b(f"b1_{l}", b1_p[l][:]) for l in range(L)]
            b2 = [_load_b(f"b2_{l}", b2_p[l][:]) for l in range(L)]
            # eps-scaled identities: z-matmul lhsT, one per layer
            epsI = []
            for l in range(L):
                t = cpool.tile([128, 128], dt.bfloat16, tag=f"epsI{l}")
                nc.vector.tensor_scalar(t[:], ident_sb[:],
                                        float(1.0 + eps_vals[l]), None,
                                        op0=OP.mult)
                epsI.append(t)

            hT = hpool.tile([F, PERP], dt.bfloat16, tag="hT")

            # ---- pre-MLP: hT = relu(relu(x Wp1 + bp1) Wp2 + bp2), transposed ----
            MLP_CHUNK = 512

            def _pre_mlp_chunks(o, end):
                while o < end:
                    cw = min(MLP_CHUNK, end - o)
                    xc = zpool.tile([F, MLP_CHUNK], dt.bfloat16, tag="xc")
                    nc.sync.dma_start(xc[:, :cw], xT_p[:, o:o + cw])
                    p1 = psmlp.tile([F, MLP_CHUNK], dt.float32, tag="mlp")
                    nc.tensor.matmul(p1[:, :cw], Wp1[:], xc[:, :cw])
                    t1 = zpool.tile([F, MLP_CHUNK], dt.bfloat16, tag="t1")
                    nc.scalar.activation(t1[:, :cw], p1[:, :cw], AF.Relu, bias=bp1[:])
                    p2 = psmlp.tile([F, MLP_CHUNK], dt.float32, tag="mlp")
                    nc.tensor.matmul(p2[:, :cw], Wp2[:], t1[:, :cw])
                    nc.scalar.activation(hT[:, o:o + cw], p2[:, :cw], AF.Relu,
                                         bias=bp2[:])
                    o += cw
                return o

            PRE_SPLIT = 13 * MLP_CHUNK  # 6656: covers table section 1 rows
            pre_o = _pre_mlp_chunks(0, PRE_SPLIT)

            pool_psums = []

            def _emit_block(b, layer_out):
                """Transpose block b of hT; DMA to ag_in (if a table is still
                needed) and accumulate pooling (if layer_out >= 1)."""
                ptr = pstr.tile([128, 128], dt.bfloat16, tag="tr")
                nc.tensor.transpose(ptr[:], hT[:, b * BLK:(b + 1) * BLK],
                                    ident_sb[:])
                hrow = epool.tile([128, 128], dt.bfloat16, tag="hrow")
                nc.scalar.activation(hrow[:], ptr[:], AF.Copy)
                if layer_out < L:
                    nc.sync.dma_start(ag_in[b * BLK:(b + 1) * BLK, :], hrow[:])
                if layer_out >= 1:
                    ohg = epool.tile([128, G], dt.bfloat16, tag="ohg")
                    nc.vector.tensor_tensor(
                        ohg[:],
                        bg_sb[:, b:b + 1].broadcast_to([128, G]),
                        iota2_sb[:, :G, 0],
                        OP.is_equal,
                    )
                    nc.tensor.matmul(
                        pool_psums[layer_out - 1][:],
                        ohg[:],
                        hrow[:],
                        start=(b == 0),
                        stop=(b == NBLK - 1),
                        skip_group_check=True,
                    )

            def _ag_chunk(l_out, lo, hi):
                """AllGather ag_in[lo:hi] into the matching contiguous table
                section (table layout is three stacked per-core sections)."""
                base = {0: 0, S1: NC * S1, S1 + S2: NC * (S1 + S2)}[lo]
                nc.gpsimd.collective_compute(
                    "AllGather", OP.bypass,
                    replica_groups=[list(range(NC))],
                    ins=[ag_in[lo:hi, :]],
                    outs=[tables[l_out][base:base + NC * (hi - lo), :]],
                )

            # emit h0 -> table0: section-1 emits + AllGather fire after only
            # the first 13 pre-MLP chunks, so the first collective (and the
            # next layer's quadrant-0/1 gathers) start early
            for b in range(S1 // BLK):
                _emit_block(b, 0)
            _ag_chunk(0, 0, S1)
            _pre_mlp_chunks(pre_o, PERP)
            for b in range(S1 // BLK, NBLK):
                _emit_block(b, 0)
                if b == (S1 + S2) // BLK - 1:
                    _ag_chunk(0, S1, S1 + S2)
            _ag_chunk(0, S1 + S2, PERP)

            # ---- GIN layers: 3-stage pipeline over rounds ----
            for l in range(L):
                pool_psums.append(pspool.tile([G, F], dt.float32, tag="pool",
                                              name=f"poolp{l}"))

                def _issue_gathers(r):
                    gts = {}
                    for (rr, q, call_off, n_slots) in call_meta:
                        if rr != r or n_slots == 0:
                            continue
                        idxs = ipool.tile([128, n_slots // 16], dt.int16,
                                          tag="idxs", name=f"idxs_l{l}_r{r}_q{q}")
                        nc.sync.dma_start(
                            idxs[:], idx_p[:, call_off // 16:(call_off + n_slots) // 16]
                        )
                        gt = gpool.tile([128, n_slots // BLK, 128], dt.bfloat16,
                                        tag="gt", name=f"gt_l{l}_r{r}_q{q}")
                        nc.gpsimd.dma_gather(
                            gt[:],
                            tables[l][q * QS:(q + 1) * QS, :],
                            idxs[:],
                            n_slots,
                            n_slots,
                            F,
                            single_packet=False,
                            queue_num=q,
                        )
                        gts[q] = (gt, call_off // BLK)
                    return gts

                def _aggregate(r, gts):
                    rblocks = [b for b in _round_blocks(r)]
                    cw = len(rblocks) * BLK
                    o = rblocks[0] * BLK
                    agg_r = psag.tile([F, RBLK * BLK], dt.float32, tag="agg",
                                      name=f"agg_l{l}_r{r}")
                    # start=True resets the whole PSUM bank, so windows can't
                    # each open their own group: zero once, all-accumulate.
                    nc.vector.memset(agg_r[:], 0.0)
                    for q in range(NQ):
                        t0 = cell_tile[(q, rblocks[0])]
                        Tr = KC * len(rblocks)
                        oh = ohpool.tile([128, 128, OHT], dt.bfloat16, tag="oh",
                                         name=f"oh_l{l}_r{r}_q{q}")
                        nc.vector.tensor_tensor(
                            oh[:, :, :Tr],
                            drel_sb[:, t0:t0 + Tr].unsqueeze(1)
                                .broadcast_to([128, 128, Tr]),
                            iota2_sb[:, :, :Tr],
                            OP.is_equal,
                        )
                        gt, c0 = gts[q]
                        for w, b in enumerate(rblocks):
                            bt = cell_tile[(q, b)]
                            for t in range(KC):
                                nc.tensor.matmul(
                                    agg_r[:, w * BLK:(w + 1) * BLK],
                                    gt[:, bt - c0 + t, :],
                                    oh[:, :, bt - t0 + t],
                                    start=False,
                                    stop=False,
                                    skip_group_check=True,
                                )
                        # spill matmuls: 512-wide one-hots over the full bank
                        ksp = int(Ksp[q, r])
                        st0 = spill_tile[(q, r)]
                        sp0 = sp_packed[(q, r)]
                        for t in range(ksp):
                            ohsp = ohsp_pool.tile([128, 512], dt.bfloat16,
                                                  tag="ohsp",
                                                  name=f"ohsp_l{l}_r{r}_q{q}_{t}")
                            nc.vector.tensor_tensor(
                                ohsp[:],
                                drsp_sb[:, sp0 + t:sp0 + t + 1]
                                    .broadcast_to([128, 512]),
                                iota512_sb[:],
                                OP.is_equal,
                            )
                            nc.tensor.matmul(
                                agg_r[:],
                                gt[:, st0 - c0 + t, :],
                                ohsp[:],
                                start=False,
                                stop=False,
                                skip_group_check=True,
                            )
                    # z = agg + (1+eps) h via eps-scaled identity matmul
                    nc.tensor.matmul(
                        agg_r[:, :cw],
                        epsI[l][:],
                        hT[:, o:o + cw],
                        start=False,
                        stop=True,
                        skip_group_check=True,
                    )
                    return (o, cw, agg_r)

                def _mlp_chunk(o, cw, agg_r):
                    # z was accumulated in PSUM: (1+eps)h + agg; copy to SBUF bf16
                    z = zpool.tile([F, MLP_CHUNK], dt.bfloat16, tag="z",
                                   name=f"z_l{l}_o{o}")
                    nc.scalar.activation(z[:, :cw], agg_r[:, :cw], AF.Copy)
                    p1 = psmlp.tile([F, MLP_CHUNK], dt.float32, tag="mlp",
                                    name=f"p1_l{l}_o{o}")
                    nc.tensor.matmul(p1[:, :cw], W1[l][:], z[:, :cw])
                    t1 = zpool.tile([F, MLP_CHUNK], dt.bfloat16, tag="t1",
                                    name=f"t1_l{l}_o{o}")
                    nc.scalar.activation(t1[:, :cw], p1[:, :cw], AF.Relu, bias=b1[l][:])
                    p2 = psmlp.tile([F, MLP_CHUNK], dt.float32, tag="mlp",
                                    name=f"p2_l{l}_o{o}")
                    nc.tensor.matmul(p2[:, :cw], W2[l][:], t1[:, :cw])
                    nc.scalar.activation(hT[:, o:o + cw], p2[:, :cw], AF.Identity,
                                         bias=b2[l][:])
                    for k in range(cw // BLK):
                        _emit_block(o // BLK + k, l + 1)

                # pipeline: gathers(r) | aggregate(r-1) | mlp+emit(r-2)
                gts_q = {}
                agg_q = {}
                ag1_round = (S1 // BLK) // RBLK        # 12: blocks 0-48 done
                ag2_round = ((S1 + S2) // BLK) // RBLK  # 18: blocks 49-73 done
                for r in range(NR + 2):
                    if r < NR:
                        gts_q[r] = _issue_gathers(r)
                    if 0 <= r - 1 < NR:
                        agg_q[r - 1] = _aggregate(r - 1, gts_q.pop(r - 1))
                    if 0 <= r - 2:
                        _mlp_chunk(*agg_q.pop(r - 2))
                        if l + 1 < L and r - 2 == ag1_round:
                            _ag_chunk(l + 1, 0, S1)
                        elif l + 1 < L and r - 2 == ag2_round:
                            _ag_chunk(l + 1, S1, S1 + S2)

                if l + 1 < L:
                    _ag_chunk(l + 1, S1 + S2, PERP)
                # extract pooled sums for this layer
                pooled_sb = epool.tile([G, F], dt.float32, tag="pooled")
                nc.scalar.activation(pooled_sb[:], pool_psums[l][:], AF.Copy)
                nc.sync.dma_start(out_p[:, l * F:(l + 1) * F], pooled_sb[:])

    nc.compile()
    return nc


def _make_in_maps(struct, inputs):
    x = np.asarray(inputs["x"], dtype=_F32)
    OHT = KC * RBLK
    iota2 = np.broadcast_to(
        np.arange(128, dtype=_F32)[:, None], (128, 128, OHT)
    ).reshape(128, 128 * OHT).astype(_BF16)  # iota2[p, j*OHT + t] = j
    iota512 = np.ascontiguousarray(np.broadcast_to(
        np.arange(512, dtype=_F32), (128, 512)))
    ident = np.eye(128, dtype=_F32).astype(_BF16)

    shared = {
        "iota2": np.ascontiguousarray(iota2),
        "iota512": iota512,
        "ident": np.ascontiguousarray(ident),
        "Wp1b": np.asarray(inputs["W_pre1"]).astype(_BF16),
        "bp1": np.asarray(inputs["b_pre1"], dtype=_F32).reshape(F, 1),
        "Wp2b": np.asarray(inputs["W_pre2"]).astype(_BF16),
        "bp2": np.asarray(inputs["b_pre2"], dtype=_F32).reshape(F, 1),
        "W1sb": np.asarray(inputs["W1s"]).astype(_BF16),
        "b1s": np.asarray(inputs["b1s"], dtype=_F32).reshape(L, F, 1),
        "W2sb": np.asarray(inputs["W2s"]).astype(_BF16),
        "b2s": np.asarray(inputs["b2s"], dtype=_F32).reshape(L, F, 1),
    }

    in_maps = []
    for c in range(NC):
        xs = np.zeros((F, PERP), dtype=_F32)
        xs[:, :PER] = x[c * PER:(c + 1) * PER].T
        m = dict(shared)
        m["xTb"] = xs.astype(_BF16)
        m["idx"] = struct["idx_planes"][c]
        m["dstrel"] = struct["drel_planes"][c]
        m["dstrelsp"] = struct["drsp_planes"][c]
        m["bg"] = struct["bg_planes"][c]
        in_maps.append(m)
    return in_maps


def kernel(**inputs):
    from concourse.bass_utils import run_bass_kernel_spmd

    edge_index = np.asarray(inputs["edge_index"])
    batch = np.asarray(inputs["batch"])
    eps = np.asarray(inputs["eps"], dtype=_F32)

    struct = _build_structures(edge_index, batch)
    nc = _build_program(struct, [float(e) for e in eps])
    in_maps = _make_in_maps(struct, inputs)

    res = run_bass_kernel_spmd(nc, in_maps, core_ids=list(range(NC)))
    out = np.zeros((G, L * F), dtype=_F32)
    for c in range(NC):
        out += res.results[c]["out"]
    return out


# revision 34
# speedup vs baseline: 1.0348x; 1.0084x over previous
"""Distributed GIN (3-layer) kernel for 8 TRN2 NeuronCores.

Sharding: nodes (and their incident in-edges) are partitioned across the 8
cores; each core keeps its node-feature shard resident in SBUF (transposed,
bf16).  Between layers the per-core shards are AllGathered into a bf16
"gather table" in DRAM; neighbor aggregation is a dma_gather (256B/edge) +
one-hot matmul segment-sum on the tensor engine.  MLP weights are replicated.
Per-graph pooled sums are computed per-core on the PE and reduced on the host.

Perf-critical structure:
- Every (quadrant, dst-block) cell gets exactly CAP=512 gather slots; overflow
  edges go to a per-(quadrant, round) spill stream aggregated with 512-wide
  one-hots.  This keeps dma_gather descriptor count (the Q7/SWDGE serial
  bottleneck) within ~7% of the true edge count.
- Gather calls are per (round, quadrant); the 4 quadrant calls run on the 4
  SWDGE queues, so their Q7 descriptor generation executes on 4 different
  core pairs concurrently (~4x the single-queue gather rate).
- 3-stage software pipeline per round r: gathers(r) / aggregation matmuls
  (r-1) / MLP+emits(r-2), with 4 rounds of gather buffering, so the PE is
  never gated on the ACT z-copy round trip and the gather stream free-runs.
- z = (1+eps)h + agg is accumulated on the PE (eps-scaled identity matmul);
  the whole MLP runs in bf16.
- The gather table is two stacked halves (all cores' rows < HALF_P, then the
  rest) so the per-layer AllGather fires as two contiguous chunks, the first
  mid-layer, overlapping the collective with compute.
"""

import math

import ml_dtypes
import numpy as np

N_NODES = 100000
N_EDGES = 1600000
F = 128              # feature dim (= hidden dim)
L = 3                # GIN layers
G = 64               # graphs
NC = 8               # NeuronCores
PER = N_NODES // NC  # 12500 real nodes per core
BLK = 128
NBLK = (PER + BLK - 1) // BLK          # 98 node blocks per core
PERP = NBLK * BLK                      # 12544 padded rows per core
TROWS = NC * PERP                      # 100352 table rows
NQ = 4
QS = TROWS // NQ                       # 25088 rows per quadrant (< int16 max)
RBLK = 4                               # dst blocks per PSUM round (1 full bank)
NR = (NBLK + RBLK - 1) // RBLK         # 25 rounds
CAP = 512                              # main slots per (q, b) cell (4 tiles)
KC = CAP // BLK                        # 4 tiles per cell
S1 = 49 * BLK                          # AllGather section 1 rows per core
S2 = 25 * BLK                          # section 2 (fires ~round 18)
S3 = PERP - S1 - S2                    # section 3 (small layer-end chunk)
HALF_P = S1                            # kept name: first-section boundary

_F32 = np.float32
_BF16 = ml_dtypes.bfloat16


def _table_row(node):
    """Global node id -> row in the (padded) all-gathered table.

    Table layout is two stacked halves so chunked AllGathers write
    contiguous slices: [all cores' rows 0:HALF_P][all cores' rows HALF_P:].
    """
    c = node // PER
    p = node % PER
    return np.where(
        p < S1, c * S1 + p,
        np.where(p < S1 + S2,
                 NC * S1 + c * S2 + (p - S1),
                 NC * (S1 + S2) + c * S3 + (p - S1 - S2)))


def _round_blocks(r):
    b0 = r * RBLK
    return range(b0, min(b0 + RBLK, NBLK))


def _build_structures(edge_index, batch):
    """Host-side preprocessing: per-core padded edge-slot streams.

    Layout per (r, q) gather call:
      [cell (q,b) : 512 slots for b in blocks(r)] ++
      [spill (q,r) : Ksp[q][r]*128 slots]
    """
    src = np.asarray(edge_index[0], dtype=np.int64)
    dst = np.asarray(edge_index[1], dtype=np.int64)
    srow = _table_row(src)
    q_of_edge = srow // QS
    qloc = (srow % QS).astype(np.int16)

    core_of_edge = dst // PER
    dst_local = dst % PER
    b_of_edge = dst_local // BLK

    # ---- per-core main/spill split (cap CAP per cell) ----
    per_core = []
    spill_counts = np.zeros((NC, NQ, NR), dtype=np.int64)
    for c in range(NC):
        sel = np.nonzero(core_of_edge == c)[0]
        eq = q_of_edge[sel]
        eb = b_of_edge[sel]
        order = np.lexsort((qloc[sel], eb, eq))
        sel = sel[order]
        eq, eb = eq[order], eb[order]
        gid = eq * NBLK + eb
        run_starts = np.zeros(NQ * NBLK + 1, dtype=np.int64)
        np.add.at(run_starts, gid + 1, 1)
        run_starts = np.cumsum(run_starts)
        rank = np.arange(gid.size) - run_starts[gid]
        is_spill = rank >= CAP
        er = eb // RBLK
        np.add.at(spill_counts, (np.full(sel.size, c)[is_spill],
                                 eq[is_spill], er[is_spill]), 1)
        per_core.append((sel, eq, eb, rank, is_spill))

    Ksp = np.ceil(spill_counts.max(axis=0) / BLK).astype(np.int64)  # [NQ, NR]

    # ---- static slot-stream layout: r-major, then q ----
    cell_tile = {}    # (q, b) -> global tile index of cell's first tile
    spill_tile = {}   # (q, r) -> global tile index of spill's first tile
    sp_packed = {}    # (q, r) -> packed spill-tile index (for the drsp plane)
    call_meta = []    # per (r, q): (slot_offset, n_slots)
    off = 0
    spk = 0
    for r in range(NR):
        for q in range(NQ):
            call_off = off
            for b in _round_blocks(r):
                cell_tile[(q, b)] = off // BLK
                off += CAP
            spill_tile[(q, r)] = off // BLK
            sp_packed[(q, r)] = spk
            off += int(Ksp[q, r]) * BLK
            spk += int(Ksp[q, r])
            call_meta.append((r, q, call_off, off - call_off))
    tot_slots = off
    tot_tiles = tot_slots // BLK
    n_sp_tiles = max(int(Ksp.sum()), 1)

    idx_planes = []
    drel_planes = []
    drsp_planes = []
    bg_planes = []
    flat_cell = np.zeros(NQ * NBLK, dtype=np.int64)
    for (q, b), t in cell_tile.items():
        flat_cell[q * NBLK + b] = t * BLK
    flat_sp = np.zeros(NQ * NR, dtype=np.int64)
    flat_sp_packed = np.zeros(NQ * NR, dtype=np.int64)
    for (q, r), t in spill_tile.items():
        flat_sp[q * NR + r] = t * BLK
        flat_sp_packed[q * NR + r] = sp_packed[(q, r)] * BLK
    for c in range(NC):
        sel, eq, eb, rank, is_spill = per_core[c]
        idx = np.zeros(tot_slots, dtype=np.int16)
        drel = np.full(tot_slots, -1.0, dtype=np.float32)
        drsp = np.full(n_sp_tiles * BLK, -1.0, dtype=np.float32)

        # main slots
        m = ~is_spill
        gid_m = eq[m] * NBLK + eb[m]
        pos_m = flat_cell[gid_m] + rank[m]
        idx[pos_m] = qloc[sel[m]]
        drel[pos_m] = (dst_local[sel[m]] - eb[m] * BLK).astype(np.float32)

        # spill slots: per (q, r) ordered by srow
        s = is_spill
        if s.any():
            eqs = eq[s]
            ers = eb[s] // RBLK
            srt = np.lexsort((qloc[sel[s]], ers, eqs))
            eqs, ers = eqs[srt], ers[srt]
            sel_s = sel[s][srt]
            gid_s = eqs * NR + ers
            rs = np.zeros(NQ * NR + 1, dtype=np.int64)
            np.add.at(rs, gid_s + 1, 1)
            rs = np.cumsum(rs)
            rank_s = np.arange(gid_s.size) - rs[gid_s]
            idx[flat_sp[gid_s] + rank_s] = qloc[sel_s]
            drsp[flat_sp_packed[gid_s] + rank_s] = (
                dst_local[sel_s] - ers * RBLK * BLK).astype(np.float32)

        idx_plane = np.tile(idx.reshape(-1, 16).T, (8, 1))          # [128, tot/16]
        drel_plane = drel.reshape(-1, BLK).T.astype(_BF16)          # [128, tot_tiles]
        drsp_plane = drsp.reshape(-1, BLK).T.astype(_F32)           # [128, n_sp_tiles]
        idx_planes.append(np.ascontiguousarray(idx_plane))
        drel_planes.append(np.ascontiguousarray(drel_plane))
        drsp_planes.append(np.ascontiguousarray(drsp_plane))

        # per-node graph ids (bf16 values; -1 pad), for on-the-fly pool one-hots
        bgfull = np.full(NBLK * BLK, -1.0, dtype=np.float32)
        bgfull[:PER] = np.asarray(batch[c * PER:(c + 1) * PER], dtype=np.float32)
        bg_planes.append(np.ascontiguousarray(
            bgfull.reshape(NBLK, BLK).T.astype(_BF16)))

    return {
        "Ksp": Ksp,
        "cell_tile": cell_tile,
        "spill_tile": spill_tile,
        "sp_packed": sp_packed,
        "call_meta": call_meta,
        "tot_slots": tot_slots,
        "tot_tiles": tot_tiles,
        "n_sp_tiles": n_sp_tiles,
        "idx_planes": idx_planes,
        "drel_planes": drel_planes,
        "drsp_planes": drsp_planes,
        "bg_planes": bg_planes,
    }


def _build_program(struct, eps_vals):
    import concourse.bacc as bacc
    import concourse.mybir as mybir
    from concourse import tile

    dt = mybir.dt
    AF = mybir.ActivationFunctionType
    OP = mybir.AluOpType

    tot_slots = struct["tot_slots"]
    tot_tiles = struct["tot_tiles"]
    n_sp_tiles = struct["n_sp_tiles"]
    call_meta = struct["call_meta"]
    cell_tile = struct["cell_tile"]
    spill_tile = struct["spill_tile"]
    sp_packed = struct["sp_packed"]
    Ksp = struct["Ksp"]
    OHT = KC * RBLK  # 16 main one-hot tiles per (q, round)

    nc = bacc.Bacc("TRN2", target_bir_lowering=False, num_swdge_queues=4)

    # ---- kernel parameters (per-core values via in_maps) ----
    xT_p = nc.declare_dram_parameter("xTb", [F, PERP], dt.bfloat16, isOutput=False)
    idx_p = nc.declare_dram_parameter("idx", [128, tot_slots // 16], dt.int16, isOutput=False)
    drel_p = nc.declare_dram_parameter("dstrel", [128, tot_tiles], dt.bfloat16, isOutput=False)
    drsp_p = nc.declare_dram_parameter("dstrelsp", [128, n_sp_tiles], dt.float32, isOutput=False)
    bg_p = nc.declare_dram_parameter("bg", [128, NBLK], dt.bfloat16, isOutput=False)
    iota2_p = nc.declare_dram_parameter("iota2", [128, 128 * OHT], dt.bfloat16, isOutput=False)
    iota512_p = nc.declare_dram_parameter("iota512", [128, 512], dt.float32, isOutput=False)
    ident_p = nc.declare_dram_parameter("ident", [128, 128], dt.bfloat16, isOutput=False)
    Wp1_p = nc.declare_dram_parameter("Wp1b", [F, F], dt.bfloat16, isOutput=False)
    bp1_p = nc.declare_dram_parameter("bp1", [F, 1], dt.float32, isOutput=False)
    Wp2_p = nc.declare_dram_parameter("Wp2b", [F, F], dt.bfloat16, isOutput=False)
    bp2_p = nc.declare_dram_parameter("bp2", [F, 1], dt.float32, isOutput=False)
    W1_p = nc.declare_dram_parameter("W1sb", [L, F, F], dt.bfloat16, isOutput=False)
    b1_p = nc.declare_dram_parameter("b1s", [L, F, 1], dt.float32, isOutput=False)
    W2_p = nc.declare_dram_parameter("W2sb", [L, F, F], dt.bfloat16, isOutput=False)
    b2_p = nc.declare_dram_parameter("b2s", [L, F, 1], dt.float32, isOutput=False)
    out_p = nc.declare_dram_parameter("out", [G, L * F], dt.float32, isOutput=True)

    # ---- internal DRAM ----
    ag_in = nc.dram_tensor("ag_in", [PERP, F], dt.bfloat16)
    tables = [
        nc.dram_tensor(f"table{l}", [TROWS, F], dt.bfloat16, addr_space="Shared")
        for l in range(L)
    ]

    with tile.TileContext(nc) as tc:
        with (
            tc.tile_pool(name="const", bufs=1) as cpool,
            tc.tile_pool(name="ht", bufs=1) as hpool,
            tc.tile_pool(name="gath", bufs=20) as gpool,
            tc.tile_pool(name="idxp", bufs=20) as ipool,
            tc.tile_pool(name="oh", bufs=12) as ohpool,
            tc.tile_pool(name="ohsp", bufs=4) as ohsp_pool,
            tc.tile_pool(name="zz", bufs=2) as zpool,
            tc.tile_pool(name="emit", bufs=4) as epool,
            tc.tile_pool(name="psag", bufs=4, space="PSUM") as psag,
            tc.tile_pool(name="psmlp", bufs=2, space="PSUM") as psmlp,
            tc.tile_pool(name="pstr", bufs=1, space="PSUM") as pstr,
            tc.tile_pool(name="pspool", bufs=1, space="PSUM") as pspool,
        ):
            # ---- load constants / weights ----
            iota2_sb = cpool.tile([128, 128, OHT], dt.bfloat16, tag="iota2")
            nc.sync.dma_start(iota2_sb[:].rearrange("p a b -> p (a b)"), iota2_p[:])
            iota512_sb = cpool.tile([128, 512], dt.float32, tag="iota512")
            nc.sync.dma_start(iota512_sb[:], iota512_p[:])
            ident_sb = cpool.tile([128, 128], dt.bfloat16, tag="ident")
            nc.sync.dma_start(ident_sb[:], ident_p[:])
            bg_sb = cpool.tile([128, NBLK], dt.bfloat16, tag="bg")
            nc.sync.dma_start(bg_sb[:], bg_p[:])
            drel_sb = cpool.tile([128, tot_tiles], dt.bfloat16, tag="drel")
            nc.sync.dma_start(drel_sb[:], drel_p[:])
            drsp_sb = cpool.tile([128, n_sp_tiles], dt.float32, tag="drsp")
            nc.sync.dma_start(drsp_sb[:], drsp_p[:])

            def _load_w(tag, pslice):
                t = cpool.tile([F, F], dt.bfloat16, tag=tag)
                nc.sync.dma_start(t[:], pslice)
                return t

            def _load_b(tag, pslice):
                t = cpool.tile([F, 1], dt.float32, tag=tag)
                nc.sync.dma_start(t[:], pslice)
                return t

            Wp1 = _load_w("Wp1", Wp1_p[:])
            Wp2 = _load_w("Wp2", Wp2_p[:])
            bp1 = _load_b("bp1", bp1_p[:])
            bp2 = _load_b("bp2", bp2_p[:])
            W1 = [_load_w(f"W1_{l}", W1_p[l][:]) for l in range(L)]
            W2 = [_load_w(f"W2_{l}", W2_p[l][:]) for l in range(L)]
            b1 = [_load_b(f"b1_{l}", b1_p[l][:]) for l in range(L)]
            b2 = [_load_b(f"b2_{l}", b2_p[l][:]) for l in range(L)]
            # eps-scaled identities: z-matmul lhsT, one per layer
            epsI = []
            for l in range(L):
                t = cpool.tile([128, 128], dt.bfloat16, tag=f"epsI{l}")
                nc.vector.tensor_scalar(t[:], ident_sb[:],
                                        float(1.0 + eps_vals[l]), None,
                                        op0=OP.mult)
                epsI.append(t)

            hT = hpool.tile([F, PERP], dt.bfloat16, tag="hT")

            # ---- pre-MLP: hT = relu(relu(x Wp1 + bp1) Wp2 + bp2), transposed ----
            MLP_CHUNK = 512

            def _pre_mlp_chunks(o, end):
                while o < end:
                    cw = min(MLP_CHUNK, end - o)
                    xc = zpool.tile([F, MLP_CHUNK], dt.bfloat16, tag="xc")
                    nc.sync.dma_start(xc[:, :cw], xT_p[:, o:o + cw])
                    p1 = psmlp.tile([F, MLP_CHUNK], dt.float32, tag="mlp")
                    nc.tensor.matmul(p1[:, :cw], Wp1[:], xc[:, :cw])
                    t1 = zpool.tile([F, MLP_CHUNK], dt.bfloat16, tag="t1")
                    nc.scalar.activation(t1[:, :cw], p1[:, :cw], AF.Relu, bias=bp1[:])
                    p2 = psmlp.tile([F, MLP_CHUNK], dt.float32, tag="mlp")
                    nc.tensor.matmul(p2[:, :cw], Wp2[:], t1[:, :cw])
                    nc.scalar.activation(hT[:, o:o + cw], p2[:, :cw], AF.Relu,
                                         bias=bp2[:])
                    o += cw
                return o

            PRE_SPLIT = 13 * MLP_CHUNK  # 6656: covers table section 1 rows
            pre_o = _pre_mlp_chunks(0, PRE_SPLIT)

            pool_psums = []

            def _emit_block(b, layer_out):
                """Transpose block b of hT; DMA to ag_in (if a table is still
                needed) and accumulate pooling (if layer_out >= 1)."""
                ptr = pstr.tile([128, 128], dt.bfloat16, tag="tr")
                nc.tensor.transpose(ptr[:], hT[:, b * BLK:(b + 1) * BLK],
                                    ident_sb[:])
                hrow = epool.tile([128, 128], dt.bfloat16, tag="hrow")
                nc.scalar.activation(hrow[:], ptr[:], AF.Copy)
                if layer_out < L:
                    nc.sync.dma_start(ag_in[b * BLK:(b + 1) * BLK, :], hrow[:])
                if layer_out >= 1:
                    ohg = epool.tile([128, G], dt.bfloat16, tag="ohg")
                    nc.vector.tensor_tensor(
                        ohg[:],
                        bg_sb[:, b:b + 1].broadcast_to([128, G]),
                        iota2_sb[:, :G, 0],
                        OP.is_equal,
                    )
                    nc.tensor.matmul(
                        pool_psums[layer_out - 1][:],
                        ohg[:],
                        hrow[:],
                        start=(b == 0),
                        stop=(b == NBLK - 1),
                        skip_group_check=True,
                    )

            def _ag_chunk(l_out, lo, hi):
                """AllGather ag_in[lo:hi] into the matching contiguous table
                section (table layout is three stacked per-core sections)."""
                base = {0: 0, S1: NC * S1, S1 + S2: NC * (S1 + S2)}[lo]
                nc.gpsimd.collective_compute(
                    "AllGather", OP.bypass,
                    replica_groups=[list(range(NC))],
                    ins=[ag_in[lo:hi, :]],
                    outs=[tables[l_out][base:base + NC * (hi - lo), :]],
                )

            # emit h0 -> table0: section-1 emits + AllGather fire after only
            # the first 13 pre-MLP chunks, so the first collective (and the
            # next layer's quadrant-0/1 gathers) start early
            for b in range(S1 // BLK):
                _emit_block(b, 0)
            _ag_chunk(0, 0, S1)
            _pre_mlp_chunks(pre_o, PERP)
            for b in range(S1 // BLK, NBLK):
                _emit_block(b, 0)
                if b == (S1 + S2) // BLK - 1:
                    _ag_chunk(0, S1, S1 + S2)
            _ag_chunk(0, S1 + S2, PERP)

            # ---- GIN layers: 3-stage pipeline over rounds ----
            for l in range(L):
                pool_psums.append(pspool.tile([G, F], dt.float32, tag="pool",
                                              name=f"poolp{l}"))

                def _issue_gathers(r):
                    gts = {}
                    calls = [cm for cm in call_meta if cm[0] == r and cm[3] > 0]
                    # one idx DMA per round: the 4 quadrant regions are
                    # contiguous in idx_p
                    off0 = calls[0][2]
                    end = calls[-1][2] + calls[-1][3]
                    idxs_r = ipool.tile([128, (end - off0) // 16], dt.int16,
                                        tag="idxs", name=f"idxs_l{l}_r{r}")
                    nc.sync.dma_start(
                        idxs_r[:], idx_p[:, off0 // 16:end // 16]
                    )
                    for (rr, q, call_off, n_slots) in calls:
                        idxs = idxs_r[:, (call_off - off0) // 16:
                                      (call_off - off0 + n_slots) // 16]
                        gt = gpool.tile([128, n_slots // BLK, 128], dt.bfloat16,
                                        tag="gt", name=f"gt_l{l}_r{r}_q{q}")
                        nc.gpsimd.dma_gather(
                            gt[:],
                            tables[l][q * QS:(q + 1) * QS, :],
                            idxs,
                            n_slots,
                            n_slots,
                            F,
                            single_packet=False,
                            queue_num=q,
                        )
                        gts[q] = (gt, call_off // BLK)
                    return gts

                def _aggregate(r, gts):
                    rblocks = [b for b in _round_blocks(r)]
                    cw = len(rblocks) * BLK
                    o = rblocks[0] * BLK
                    agg_r = psag.tile([F, RBLK * BLK], dt.float32, tag="agg",
                                      name=f"agg_l{l}_r{r}")
                    # start=True resets the whole PSUM bank, so windows can't
                    # each open their own group: zero once, all-accumulate.
                    nc.vector.memset(agg_r[:], 0.0)
                    for q in range(NQ):
                        t0 = cell_tile[(q, rblocks[0])]
                        Tr = KC * len(rblocks)
                        oh = ohpool.tile([128, 128, OHT], dt.bfloat16, tag="oh",
                                         name=f"oh_l{l}_r{r}_q{q}")
                        nc.vector.tensor_tensor(
                            oh[:, :, :Tr],
                            drel_sb[:, t0:t0 + Tr].unsqueeze(1)
                                .broadcast_to([128, 128, Tr]),
                            iota2_sb[:, :, :Tr],
                            OP.is_equal,
                        )
                        gt, c0 = gts[q]
                        for w, b in enumerate(rblocks):
                            bt = cell_tile[(q, b)]
                            for t in range(KC):
                                nc.tensor.matmul(
                                    agg_r[:, w * BLK:(w + 1) * BLK],
                                    gt[:, bt - c0 + t, :],
                                    oh[:, :, bt - t0 + t],
                                    start=False,
                                    stop=False,
                                    skip_group_check=True,
                                )
                        # spill matmuls: 512-wide one-hots over the full bank
                        ksp = int(Ksp[q, r])
                        st0 = spill_tile[(q, r)]
                        sp0 = sp_packed[(q, r)]
                        for t in range(ksp):
                            ohsp = ohsp_pool.tile([128, 512], dt.bfloat16,
                                                  tag="ohsp",
                                                  name=f"ohsp_l{l}_r{r}_q{q}_{t}")
                            nc.vector.tensor_tensor(
                                ohsp[:],
                                drsp_sb[:, sp0 + t:sp0 + t + 1]
                                    .broadcast_to([128, 512]),
                                iota512_sb[:],
                                OP.is_equal,
                            )
                            nc.tensor.matmul(
                                agg_r[:],
                                gt[:, st0 - c0 + t, :],
                                ohsp[:],
                                start=False,
                                stop=False,
                                skip_group_check=True,
                            )
                    # z = agg + (1+eps) h via eps-scaled identity matmul
                    nc.tensor.matmul(
                        agg_r[:, :cw],
                        epsI[l][:],
                        hT[:, o:o + cw],
                        start=False,
                        stop=True,
                        skip_group_check=True,
                    )
                    return (o, cw, agg_r)

                def _mlp_chunk(o, cw, agg_r):
                    # z was accumulated in PSUM: (1+eps)h + agg; copy to SBUF bf16
                    z = zpool.tile([F, MLP_CHUNK], dt.bfloat16, tag="z",
                                   name=f"z_l{l}_o{o}")
                    nc.scalar.activation(z[:, :cw], agg_r[:, :cw], AF.Copy)
                    p1 = psmlp.tile([F, MLP_CHUNK], dt.float32, tag="mlp",
                                    name=f"p1_l{l}_o{o}")
                    nc.tensor.matmul(p1[:, :cw], W1[l][:], z[:, :cw])
                    t1 = zpool.tile([F, MLP_CHUNK], dt.bfloat16, tag="t1",
                                    name=f"t1_l{l}_o{o}")
                    nc.scalar.activation(t1[:, :cw], p1[:, :cw], AF.Relu, bias=b1[l][:])
                    p2 = psmlp.tile([F, MLP_CHUNK], dt.float32, tag="mlp",
                                    name=f"p2_l{l}_o{o}")
                    nc.tensor.matmul(p2[:, :cw], W2[l][:], t1[:, :cw])
                    nc.scalar.activation(hT[:, o:o + cw], p2[:, :cw], AF.Identity,
                                         bias=b2[l][:])
                    for k in range(cw // BLK):
                        _emit_block(o // BLK + k, l + 1)

                # pipeline: gathers(r) | aggregate(r-1) | mlp+emit(r-2)
                gts_q = {}
                agg_q = {}
                ag1_round = (S1 // BLK) // RBLK        # 12: blocks 0-48 done
                ag2_round = ((S1 + S2) // BLK) // RBLK  # 18: blocks 49-73 done
                for r in range(NR + 2):
                    if r < NR:
                        gts_q[r] = _issue_gathers(r)
                    if 0 <= r - 1 < NR:
                        agg_q[r - 1] = _aggregate(r - 1, gts_q.pop(r - 1))
                    if 0 <= r - 2:
                        _mlp_chunk(*agg_q.pop(r - 2))
                        if l + 1 < L and r - 2 == ag1_round:
                            _ag_chunk(l + 1, 0, S1)
                        elif l + 1 < L and r - 2 == ag2_round:
                            _ag_chunk(l + 1, S1, S1 + S2)

                if l + 1 < L:
                    _ag_chunk(l + 1, S1 + S2, PERP)
                # extract pooled sums for this layer
                pooled_sb = epool.tile([G, F], dt.float32, tag="pooled")
                nc.scalar.activation(pooled_sb[:], pool_psums[l][:], AF.Copy)
                nc.sync.dma_start(out_p[:, l * F:(l + 1) * F], pooled_sb[:])

    nc.compile()
    return nc


def _make_in_maps(struct, inputs):
    x = np.asarray(inputs["x"], dtype=_F32)
    OHT = KC * RBLK
    iota2 = np.broadcast_to(
        np.arange(128, dtype=_F32)[:, None], (128, 128, OHT)
    ).reshape(128, 128 * OHT).astype(_BF16)  # iota2[p, j*OHT + t] = j
    iota512 = np.ascontiguousarray(np.broadcast_to(
        np.arange(512, dtype=_F32), (128, 512)))
    ident = np.eye(128, dtype=_F32).astype(_BF16)

    shared = {
        "iota2": np.ascontiguousarray(iota2),
        "iota512": iota512,
        "ident": np.ascontiguousarray(ident),
        "Wp1b": np.asarray(inputs["W_pre1"]).astype(_BF16),
        "bp1": np.asarray(inputs["b_pre1"], dtype=_F32).reshape(F, 1),
        "Wp2b": np.asarray(inputs["W_pre2"]).astype(_BF16),
        "bp2": np.asarray(inputs["b_pre2"], dtype=_F32).reshape(F, 1),
        "W1sb": np.asarray(inputs["W1s"]).astype(_BF16),
        "b1s": np.asarray(inputs["b1s"], dtype=_F32).reshape(L, F, 1),
        "W2sb": np.asarray(inputs["W2s"]).astype(_BF16),
        "b2s": np.asarray(inputs["b2s"], dtype=_F32).reshape(L, F, 1),
    }

    in_maps = []
    for c in range(NC):
        xs = np.zeros((F, PERP), dtype=_F32)
        xs[:, :PER] = x[c * PER:(c + 1) * PER].T
        m = dict(shared)
        m["xTb"] = xs.astype(_BF16)
        m["idx"] = struct["idx_planes"][c]
        m["dstrel"] = struct["drel_planes"][c]
        m["dstrelsp"] = struct["drsp_planes"][c]
        m["bg"] = struct["bg_planes"][c]
        in_maps.append(m)
    return in_maps


def kernel(**inputs):
    from concourse.bass_utils import run_bass_kernel_spmd

    edge_index = np.asarray(inputs["edge_index"])
    batch = np.asarray(inputs["batch"])
    eps = np.asarray(inputs["eps"], dtype=_F32)

    struct = _build_structures(edge_index, batch)
    nc = _build_program(struct, [float(e) for e in eps])
    in_maps = _make_in_maps(struct, inputs)

    res = run_bass_kernel_spmd(nc, in_maps, core_ids=list(range(NC)))
    out = np.zeros((G, L * F), dtype=_F32)
    for c in range(NC):
        out += res.results[c]["out"]
    return out
